# revision 1
# baseline (speedup 1.0000x reference)
"""BiGraphSAGEDecoder Trainium2 kernel.

Sharding: 8 cores = 4 batches x {up-path, down-path}. One SPMD bass program;
the up/down asymmetry is handled purely by data (down cores receive host-
transposed adjacency / adjacency-weight matrices). Per layer, the two cores of
a batch exchange their halves of the concatenated features with a 2-rank
AllGather, then each normalizes redundantly so both hold the full h.

Math per layer (per core, its path):
  prod = adj . (Wadj . mask + unmask)            (elementwise, DVE)
  s    = prod^T @ h                              (PE, lhsT = prod blocks)
  xT   = (inv @ s)^T                             (PE, rhs = invT streamed)
  cat_own = [x @ Wcat | h @ Wbias_half]          (PE; h@Wb via PE-transposed h)
  exchange cat halves -> full cat; h' = lrelu(cat / max(||cat||, 1e-12))
Layer 2 computes only the two drug rows after s. Head: bilinear form on PE.
"""

import os
import sys
import types
import contextlib

sys.path.insert(0, "/opt/trn_rl_repo")

import numpy as np

import concourse.bass as bass
import concourse.tile as tile
from concourse import mybir, bacc
from concourse.mybir import AxisListType
from concourse.masks import make_identity
from concourse.bass_utils import run_bass_kernel_spmd

FP = mybir.dt.float32
FPR = mybir.dt.float32r
AF = mybir.ActivationFunctionType
ALU = mybir.AluOpType

# ---------------------------------------------------------------------------
# Environment patches (required for this container's toolchain)
# ---------------------------------------------------------------------------


def install_ntff_shim():
    """antenv.axon_hooks is absent in this image; provide it so trace=True
    profiling works (used by test.py, harmless otherwise)."""
    try:
        import antenv.axon_hooks  # noqa: F401
        return
    except ImportError:
        pass
    try:
        import antenv
    except ImportError:
        return
    mod = types.ModuleType("antenv.axon_hooks")
    _holder = {"hook": None}
    mod.set_axon_ntff_profile_hook = lambda h: _holder.__setitem__("hook", h)
    mod.get_axon_ntff_profile_hook = lambda: _holder["hook"]
    sys.modules["antenv.axon_hooks"] = mod
    antenv.axon_hooks = mod
    try:
        from trn_agent_boot.trn_boot import _ntff_profile_via_ctypes

        hook = _ntff_profile_via_ctypes("/opt/axon/libaxon_pjrt.so")
        if hook is not None:
            mod.set_axon_ntff_profile_hook(hook)
    except Exception:
        pass


install_ntff_shim()

if os.environ.get("KGSD_LDW_OPT", "1") != "0":
    # experiment: let walrus dedup back-to-back LDWEIGHTS
    import concourse.bass_utils as _bu
    _orig_run_command = _bu.run_command

    def _patched_run_command(argv, **kw):
        argv = ["--enable-ldw-opt=true" if a == "--enable-ldw-opt=false"
                else a for a in argv]
        return _orig_run_command(argv, **kw)

    _bu.run_command = _patched_run_command

# ---------------------------------------------------------------------------
# Problem constants
# ---------------------------------------------------------------------------

N_FULL = 2048
B = 4
P = 128
DOUT = 256     # per-path cat chunk width
BH = 128       # bias half width per core
DEC = 128
DINS = (256, 768, 768)   # per-layer input dims
EPS = 1e-12
LEAK = 0.1

JSB = 256      # mm1 column superblock (j columns per packed strip tile)
KPACK = 2      # k-tiles packed per mm1 strip tile


def _ceil_div(a, b):
    return -(-a // b)


# ---------------------------------------------------------------------------
# Program builder
# ---------------------------------------------------------------------------

class _StopBuild(Exception):
    pass


def build_program(n_cores: int, N: int = N_FULL, stop_phase: int = 99):
    """Build the SPMD bass program. Returns (nc, input_names).

    stop_phase (debug): 1=x-load only, 2=+l0 bias, 3=+l0 mm1, 4=+l0 mm2,
    5=+l0 mm3+AG+assemble, 6=+l1, 7=full.
    """
    NT = N // P                # k/j/i tiles of 128
    NPAIR = NT // KPACK        # packed k strip-pairs
    NJSB = N // JSB            # mm1 column superblocks
    MM2_JP = 512               # mm2 j' superblock width
    NJP = N // MM2_JP

    nc = bacc.Bacc("TRN2", target_bir_lowering=False, debug=False,
                   num_devices=n_cores)

    # --- DRAM I/O ---
    x_d = nc.dram_tensor("x", [N, DINS[0]], FP, kind="ExternalInput")
    adj_d = nc.dram_tensor("adj", [N, N], FP, kind="ExternalInput")
    invT_d = nc.dram_tensor("invT", [N, N], FP, kind="ExternalInput")
    wa_d = [nc.dram_tensor(f"w{l}a", [N, N], FP, kind="ExternalInput")
            for l in range(3)]
    wc_d = [nc.dram_tensor(f"w{l}c", [DINS[l], DOUT], FP, kind="ExternalInput")
            for l in range(3)]
    wb_d = [nc.dram_tensor(f"w{l}b", [DINS[l], BH], FP, kind="ExternalInput")
            for l in range(3)]
    p1_d = nc.dram_tensor("p1", [3 * DOUT, DEC], FP, kind="ExternalInput")
    p2_d = nc.dram_tensor("p2", [DEC, DEC], FP, kind="ExternalInput")
    y_d = nc.dram_tensor("ypred", [1, 1], FP, kind="ExternalOutput")

    groups = [[i, i + 1] for i in range(0, n_cores, 2)]

    try:
      with tile.TileContext(nc) as tc:
        with contextlib.ExitStack() as ctx:
            # --- pools (all opened once; tags bound memory) ---
            const_p = ctx.enter_context(tc.tile_pool(name="const", bufs=1))
            h_p = ctx.enter_context(tc.tile_pool(name="h", bufs=1))
            # s and hT share one 48KB/partition slot (disjoint lifetimes:
            # hT(l) dies before s(l) is written; s(l) dies before hT(l+1))
            s_p = ctx.enter_context(tc.tile_pool(name="s", bufs=1))
            adj_p = ctx.enter_context(tc.tile_pool(name="adjs", bufs=3))
            w_p = ctx.enter_context(tc.tile_pool(name="ws", bufs=3))
            prod_p = ctx.enter_context(tc.tile_pool(name="prod", bufs=10))
            inv_p = ctx.enter_context(tc.tile_pool(name="invs", bufs=3))
            mm3l_p = ctx.enter_context(tc.tile_pool(name="mm3l", bufs=4))
            wcb_p = ctx.enter_context(tc.tile_pool(name="wcb", bufs=2))
            misc_p = ctx.enter_context(tc.tile_pool(name="misc", bufs=3))
            norm_p = ctx.enter_context(tc.tile_pool(name="norm", bufs=3))
            psum_p = ctx.enter_context(
                tc.tile_pool(name="psum", bufs=8, space="PSUM"))
            dram_p = ctx.enter_context(
                tc.tile_pool(name="dram", bufs=2, space="DRAM"))

            ident = const_p.tile([P, P], FP, tag="ident")
            make_identity(nc, ident)

            # h as per-row-block tiles (pipelines assembly/normalize/use)
            def new_h(din):
                return [h_p.tile([P, din], FPR, tag=f"h{kt}", name="h_t")
                        for kt in range(NT)]

            h_t = new_h(DINS[0])
            # load x -> h tiles
            for kt in range(NT):
                nc.sync.dma_start(
                    h_t[kt][:],
                    x_d.ap()[kt * P:(kt + 1) * P, :].bitcast(FPR))

            drug_rows = None  # final [2,768] tile

            def _dump_and_done(src_ap):
                y_sb0 = misc_p.tile([1, 1], FP, tag="y_sb", name="y_dbg")
                nc.vector.tensor_copy(y_sb0[:], src_ap)
                nc.sync.dma_start(y_d.ap(), y_sb0[:])

            if stop_phase <= 1:
                _dump_and_done(h_t[0][0:1, 0:1])
            n_layers = 0 if stop_phase <= 1 else (
                1 if stop_phase <= 5 else (2 if stop_phase <= 6 else 3))
            for l in range(n_layers):
                din = DINS[l]
                ND = din // P
                last = (l == 2)

                # ---- weights for this layer ----
                wc_t = wcb_p.tile([P, ND * DOUT], FPR, tag="wc")
                for d in range(ND):
                    nc.scalar.dma_start(
                        wc_t[:, d * DOUT:(d + 1) * DOUT],
                        wc_d[l].ap()[d * P:(d + 1) * P, :].bitcast(FPR))
                wb_t = wcb_p.tile([P, ND * BH], FP, tag="wb")
                for d in range(ND):
                    nc.scalar.dma_start(
                        wb_t[:, d * BH:(d + 1) * BH],
                        wb_d[l].ap()[d * P:(d + 1) * P, :])

                # ---- bias chunk: hT = h^T (PE), bias = h @ Wb_half ----
                if not last:
                    # stage DRAM for own cat chunk, split in row halves so
                    # each half's AllGather can overlap the other's compute
                    stage_h = [dram_p.tile([N // 2, DOUT + BH], FP,
                                           tag=f"stage{hh}", name="stage_h")
                               for hh in range(2)]
                    hT_t = s_p.tile([P, ND * N], FP, tag="s")
                    for d in range(ND):
                        for it in range(NT):
                            pt = psum_p.tile([P, P], FP, tag="ps")
                            nc.tensor.transpose(
                                pt[:],
                                h_t[it][:, d * P:(d + 1) * P].bitcast(FP),
                                ident[:])
                            dst = hT_t[:, d * N + it * P: d * N + (it + 1) * P]
                            if it % 2 == 0:
                                nc.vector.tensor_copy(dst, pt[:])
                            else:
                                nc.scalar.copy(dst, pt[:])
                    for it in range(NT):
                        pb = psum_p.tile([P, BH], FP, tag="ps")
                        for d in range(ND):
                            nc.tensor.matmul(
                                pb[:],
                                hT_t[:, d * N + it * P: d * N + (it + 1) * P],
                                wb_t[:, d * BH:(d + 1) * BH],
                                start=(d == 0), stop=(d == ND - 1))
                        sb = misc_p.tile([P, BH], FP, tag="stg_b")
                        nc.scalar.copy(sb[:], pb[:])
                        hh, io = divmod(it, NT // 2)
                        nc.scalar.dma_start(
                            stage_h[hh][io * P:(io + 1) * P, DOUT:DOUT + BH],
                            sb[:])
                else:
                    # only rows N-2, N-1 (partition-0 copy built by l1)
                    hTd = misc_p.tile([P, ND * 2], FP, tag="hTd")
                    for d in range(ND):
                        pt = psum_p.tile([P, 2], FP, tag="ps")
                        nc.tensor.transpose(
                            pt[:],
                            hdrug_t[:, d * P:(d + 1) * P],
                            ident[0:2, 0:2])
                        nc.vector.tensor_copy(hTd[:, d * 2:(d + 1) * 2], pt[:])
                    pb = psum_p.tile([2, BH], FP, tag="ps")
                    for d in range(ND):
                        nc.tensor.matmul(
                            pb[:], hTd[:, d * 2:(d + 1) * 2],
                            wb_t[:, d * BH:(d + 1) * BH],
                            start=(d == 0), stop=(d == ND - 1))
                    stage2 = dram_p.tile([2, DOUT + BH], FP, tag="stage2")
                    sb = misc_p.tile([2, BH], FP, tag="stg_b")
                    nc.scalar.copy(sb[:], pb[:])
                    nc.scalar.dma_start(stage2[:, DOUT:DOUT + BH], sb[:])

                if stop_phase <= 2 and l == 0:
                    _dump_and_done(h_t[0][0:1, 0:1])
                    break

                # ---- mm1: s = prod^T @ h ----
                s_t = s_p.tile([P, NT * din], FPR, tag="s")
                for jsb in range(NJSB):
                    prods = []
                    for t in range(NPAIR):
                        a_t = adj_p.tile([P, KPACK, JSB], FP, tag="adj")
                        nc.sync.dma_start(
                            a_t[:],
                            adj_d.ap()[t * KPACK * P:(t + 1) * KPACK * P,
                                       jsb * JSB:(jsb + 1) * JSB]
                            .rearrange("(a p) c -> p a c", p=P))
                        ww_t = w_p.tile([P, KPACK, JSB], FP, tag="wadj")
                        nc.scalar.dma_start(
                            ww_t[:],
                            wa_d[l].ap()[t * KPACK * P:(t + 1) * KPACK * P,
                                         jsb * JSB:(jsb + 1) * JSB]
                            .rearrange("(a p) c -> p a c", p=P))
                        # mask already baked into Wadj host-side
                        pr_t = prod_p.tile([P, KPACK, JSB], FPR, tag="prod")
                        nc.vector.tensor_tensor(pr_t[:], a_t[:], ww_t[:],
                                                ALU.mult)
                        prods.append(pr_t)
                    for jl in range(JSB // P):
                        j = jsb * (JSB // P) + jl
                        pA = psum_p.tile([P, min(din, 512)], FP, tag="ps")
                        pBw = din - 512
                        pB = psum_p.tile([P, pBw], FP, tag="ps", name="pB") \
                            if pBw > 0 else None
                        for t in range(NPAIR):
                            for a in range(KPACK):
                                k = t * KPACK + a
                                lhsT = prods[t][:, a, jl * P:(jl + 1) * P]
                                st = (k == 0)
                                sp = (k == NT - 1)
                                nc.tensor.matmul(
                                    pA[:], lhsT,
                                    h_t[k][:, 0:min(din, 512)],
                                    start=st, stop=sp)
                                if pB is not None:
                                    nc.tensor.matmul(
                                        pB[:], lhsT,
                                        h_t[k][:, 512:din],
                                        start=st, stop=sp)
                        eng = nc.scalar if (j % 2 == 0) else nc.vector
                        if eng is nc.scalar:
                            nc.scalar.copy(
                                s_t[:, j * din: j * din + min(din, 512)], pA[:])
                            if pB is not None:
                                nc.scalar.copy(
                                    s_t[:, j * din + 512:(j + 1) * din], pB[:])
                        else:
                            nc.vector.tensor_copy(
                                s_t[:, j * din: j * din + min(din, 512)], pA[:])
                            if pB is not None:
                                nc.vector.tensor_copy(
                                    s_t[:, j * din + 512:(j + 1) * din], pB[:])

                if stop_phase <= 3 and l == 0:
                    _dump_and_done(s_t[0:1, 0:1])
                    break

                # ---- mm2: xT = (inv @ s)^T ----
                if not last:
                    xT_dram = dram_p.tile([din, N], FP, tag="xT")
                    for jp in range(NJP):
                        pxs = [psum_p.tile([P, MM2_JP], FP, tag="ps", name="px")
                               for _ in range(ND)]
                        for jt in range(NT):
                            r_t = inv_p.tile([P, MM2_JP], FPR, tag="inv")
                            nc.sync.dma_start(
                                r_t[:],
                                invT_d.ap()[jt * P:(jt + 1) * P,
                                            jp * MM2_JP:(jp + 1) * MM2_JP]
                                .bitcast(FPR))
                            for d in range(ND):
                                nc.tensor.matmul(
                                    pxs[d][:],
                                    s_t[:, jt * din + d * P:
                                        jt * din + (d + 1) * P],
                                    r_t[:],
                                    start=(jt == 0), stop=(jt == NT - 1))
                        for d in range(ND):
                            xs = misc_p.tile([P, MM2_JP], FPR, tag="xstg",
                                             bufs=4)
                            if d % 2 == 0:
                                nc.scalar.copy(xs[:], pxs[d][:])
                            else:
                                nc.vector.tensor_copy(xs[:], pxs[d][:])
                            nc.sync.dma_start(
                                xT_dram[d * P:(d + 1) * P,
                                        jp * MM2_JP:(jp + 1) * MM2_JP]
                                .bitcast(FPR),
                                xs[:])

                    if stop_phase <= 4 and l == 0:
                        _dump_and_done(s_t[0:1, 0:1])
                        break

                    # ---- mm3: cat_own = x @ Wc ----
                    for it in range(NT):
                        pc = psum_p.tile([P, DOUT], FP, tag="ps")
                        for d in range(ND):
                            lt = mm3l_p.tile([P, P], FPR, tag="mm3l")
                            nc.sync.dma_start(
                                lt[:],
                                xT_dram[d * P:(d + 1) * P,
                                        it * P:(it + 1) * P].bitcast(FPR))
                            nc.tensor.matmul(
                                pc[:], lt[:],
                                wc_t[:, d * DOUT:(d + 1) * DOUT],
                                start=(d == 0), stop=(d == ND - 1))
                        sc = misc_p.tile([P, DOUT], FP, tag="stg_c")
                        nc.scalar.copy(sc[:], pc[:])
                        hh, io = divmod(it, NT // 2)
                        nc.scalar.dma_start(
                            stage_h[hh][io * P:(io + 1) * P, 0:DOUT], sc[:])

                    if stop_phase <= 4.3 and l == 0:
                        _dump_and_done(s_t[0:1, 0:1])
                        break

                    # ---- exchange (two halves, overlapped) ----
                    ag_h = []
                    for hh in range(2):
                        agt = dram_p.tile([2, N // 2, DOUT + BH], FP,
                                          tag=f"ag{hh}", name="ag_h")
                        nc.gpsimd.collective_compute(
                            "AllGather", ALU.bypass, replica_groups=groups,
                            ins=[stage_h[hh].opt()], outs=[agt.opt()])
                        ag_h.append(agt)

                    if stop_phase <= 4.6 and l == 0:
                        _dump_and_done(s_t[0:1, 0:1])
                        break

                    # ---- assemble + normalize + lrelu -> new h ----
                    dnext = 3 * DOUT
                    h_t = new_h(dnext)
                    for it in range(NT):
                        hh, io = divmod(it, NT // 2)
                        ag = ag_h[hh]
                        sl = slice(io * P, (io + 1) * P)
                        ht = h_t[it]
                        nc.sync.dma_start(
                            ht[:, 0:DOUT], ag[0, sl, 0:DOUT].bitcast(FPR))
                        nc.sync.dma_start(
                            ht[:, DOUT:2 * DOUT],
                            ag[1, sl, 0:DOUT].bitcast(FPR))
                        nc.sync.dma_start(
                            ht[:, 2 * DOUT:2 * DOUT + BH],
                            ag[0, sl, DOUT:DOUT + BH].bitcast(FPR))
                        nc.sync.dma_start(
                            ht[:, 2 * DOUT + BH:3 * DOUT],
                            ag[1, sl, DOUT:DOUT + BH].bitcast(FPR))
                        ct = ht[:]
                        if stop_phase <= 4.7 and l == 0:
                            continue
                        sq = norm_p.tile([P, dnext], FPR, tag="sq")
                        ssq = norm_p.tile([P, 1], FP, tag="ssq")
                        nc.vector.tensor_tensor(sq[:], ct, ct, ALU.mult)
                        nc.vector.tensor_reduce(ssq[:], sq[:],
                                                AxisListType.X, ALU.add)
                        if stop_phase <= 4.75 and l == 0:
                            continue
                        nrm = norm_p.tile([P, 1], FP, tag="nrm")
                        nc.scalar.activation(nrm[:], ssq[:], AF.Sqrt)
                        nc.vector.tensor_scalar_max(nrm[:], nrm[:], EPS)
                        rn = norm_p.tile([P, 1], FP, tag="rn")
                        nc.vector.reciprocal(rn[:], nrm[:])
                        if stop_phase <= 4.8 and l == 0:
                            continue
                        # h = max(x, 0.1x) with x = cat/norm  (leaky relu)
                        nc.vector.tensor_scalar(ct, ct, rn[:], None, ALU.mult)
                        if stop_phase <= 4.85 and l == 0:
                            continue
                        nc.scalar.mul(sq[:], ct, LEAK)
                        nc.vector.tensor_max(ct, ct, sq[:])
                    if l == 1:
                        # extra partition-0-based copy of the two drug rows
                        # (PE ops cannot address partitions 126:128)
                        hdrug_t = misc_p.tile([2, dnext], FP, tag="hdrug", bufs=1)
                        agl = ag_h[1]
                        NH = N // 2
                        nc.sync.dma_start(hdrug_t[:, 0:DOUT],
                                          agl[0, NH - 2:NH, 0:DOUT])
                        nc.sync.dma_start(hdrug_t[:, DOUT:2 * DOUT],
                                          agl[1, NH - 2:NH, 0:DOUT])
                        nc.sync.dma_start(hdrug_t[:, 2 * DOUT:2 * DOUT + BH],
                                          agl[0, NH - 2:NH, DOUT:DOUT + BH])
                        nc.sync.dma_start(hdrug_t[:, 2 * DOUT + BH:3 * DOUT],
                                          agl[1, NH - 2:NH, DOUT:DOUT + BH])
                        dsq = norm_p.tile([2, dnext], FP, tag="sq")
                        dssq = norm_p.tile([2, 1], FP, tag="ssq")
                        nc.vector.tensor_tensor(dsq[:], hdrug_t[:],
                                                hdrug_t[:], ALU.mult)
                        nc.vector.tensor_reduce(dssq[:], dsq[:],
                                                AxisListType.X, ALU.add)
                        dnrm = norm_p.tile([2, 1], FP, tag="nrm")
                        nc.scalar.activation(dnrm[:], dssq[:], AF.Sqrt)
                        nc.vector.tensor_scalar_max(dnrm[:], dnrm[:], EPS)
                        drn = norm_p.tile([2, 1], FP, tag="rn")
                        nc.vector.reciprocal(drn[:], dnrm[:])
                        nc.vector.tensor_scalar(hdrug_t[:], hdrug_t[:],
                                                drn[:], None, ALU.mult)
                        nc.scalar.mul(dsq[:], hdrug_t[:], LEAK)
                        nc.vector.tensor_max(hdrug_t[:], hdrug_t[:], dsq[:])
                else:
                    # ---- l2: only drug rows j' in {N-2, N-1} ----
                    xT2 = misc_p.tile([P, ND * 2], FP, tag="xT2")
                    px2 = [psum_p.tile([P, 2], FP, tag="ps", name="px2")
                           for _ in range(ND)]
                    for jt in range(NT):
                        r_t = inv_p.tile([P, 2], FPR, tag="inv2")
                        nc.sync.dma_start(
                            r_t[:],
                            invT_d.ap()[jt * P:(jt + 1) * P, N - 2:N]
                            .bitcast(FPR))
                        for d in range(ND):
                            nc.tensor.matmul(
                                px2[d][:],
                                s_t[:, jt * din + d * P:jt * din + (d + 1) * P],
                                r_t[:], start=(jt == 0), stop=(jt == NT - 1))
                    for d in range(ND):
                        nc.vector.tensor_copy(xT2[:, d * 2:(d + 1) * 2],
                                              px2[d][:])
                    pc = psum_p.tile([2, DOUT], FP, tag="ps")
                    for d in range(ND):
                        nc.tensor.matmul(
                            pc[:], xT2[:, d * 2:(d + 1) * 2],
                            wc_t[:, d * DOUT:(d + 1) * DOUT].bitcast(FP),
                            start=(d == 0), stop=(d == ND - 1))
                    sc = misc_p.tile([2, DOUT], FP, tag="stg_c")
                    nc.scalar.copy(sc[:], pc[:])
                    nc.scalar.dma_start(stage2[:, 0:DOUT], sc[:])

                    ag2 = dram_p.tile([2, 2, DOUT + BH], FP, tag="ag2")
                    nc.gpsimd.collective_compute(
                        "AllGather", ALU.bypass, replica_groups=groups,
                        ins=[stage2.opt()], outs=[ag2.opt()])

                    dnext = 3 * DOUT
                    dr = norm_p.tile([2, dnext], FP, tag="drug", bufs=1)
                    nc.sync.dma_start(dr[:, 0:DOUT], ag2[0, :, 0:DOUT])
                    nc.sync.dma_start(dr[:, DOUT:2 * DOUT],
                                      ag2[1, :, 0:DOUT])
                    nc.sync.dma_start(dr[:, 2 * DOUT:2 * DOUT + BH],
                                      ag2[0, :, DOUT:DOUT + BH])
                    nc.sync.dma_start(dr[:, 2 * DOUT + BH:3 * DOUT],
                                      ag2[1, :, DOUT:DOUT + BH])
                    sq = norm_p.tile([2, dnext], FP, tag="sq")
                    ssq = norm_p.tile([2, 1], FP, tag="ssq")
                    nc.vector.tensor_tensor(sq[:], dr[:], dr[:], ALU.mult)
                    nc.vector.tensor_reduce(ssq[:], sq[:],
                                            AxisListType.X, ALU.add)
                    nrm = norm_p.tile([2, 1], FP, tag="nrm")
                    nc.scalar.activation(nrm[:], ssq[:], AF.Sqrt)
                    nc.vector.tensor_scalar_max(nrm[:], nrm[:], EPS)
                    rn = norm_p.tile([2, 1], FP, tag="rn")
                    nc.vector.reciprocal(rn[:], nrm[:])
                    nc.vector.tensor_scalar(dr[:], dr[:], rn[:], None,
                                            ALU.mult)
                    nc.scalar.mul(sq[:], dr[:], LEAK)
                    nc.vector.tensor_max(dr[:], dr[:], sq[:])
                    drug_rows = dr

            if stop_phase <= 6:
                if stop_phase >= 5 and n_layers >= 1:
                    _dump_and_done(h_t[0][0:1, 0:1])
            do_head = stop_phase > 6
            # ---- head: ypred = (a P1 P2) . (b P1) ----
            D3 = 3 * DOUT
            ND3 = D3 // P
            if do_head:
                p1_t = const_p.tile([P, ND3 * DEC], FP, tag="p1")
                for d in range(ND3):
                    nc.sync.dma_start(p1_t[:, d * DEC:(d + 1) * DEC],
                                      p1_d.ap()[d * P:(d + 1) * P, :])
                p2_t = const_p.tile([P, DEC], FP, tag="p2")
                nc.sync.dma_start(p2_t[:], p2_d.ap())
            if do_head:
                dT = misc_p.tile([P, ND3 * 2], FP, tag="dT")
                for d in range(ND3):
                    pt = psum_p.tile([P, 2], FP, tag="ps")
                    nc.tensor.transpose(pt[:], drug_rows[:, d * P:(d + 1) * P],
                                        ident[0:2, 0:2])
                    nc.vector.tensor_copy(dT[:, d * 2:(d + 1) * 2], pt[:])
                pw = psum_p.tile([P, 2], FP, tag="ps")
                for d in range(ND3):
                    nc.tensor.matmul(pw[:], p1_t[:, d * DEC:(d + 1) * DEC],
                                     dT[:, d * 2:(d + 1) * 2],
                                     start=(d == 0), stop=(d == ND3 - 1))
                w_sb = misc_p.tile([P, 2], FP, tag="w_sb")
                nc.vector.tensor_copy(w_sb[:], pw[:])
                ptt = psum_p.tile([P, 1], FP, tag="ps")
                nc.tensor.matmul(ptt[:], p2_t[:], w_sb[:, 0:1], start=True,
                                 stop=True)
                t_sb = misc_p.tile([P, 1], FP, tag="t_sb")
                nc.vector.tensor_copy(t_sb[:], ptt[:])
                py = psum_p.tile([1, 1], FP, tag="ps")
                nc.tensor.matmul(py[:], t_sb[:], w_sb[:, 1:2], start=True,
                                 stop=True)
                y_sb = misc_p.tile([1, 1], FP, tag="y_sb")
                nc.vector.tensor_copy(y_sb[:], py[:])
                nc.sync.dma_start(y_d.ap(), y_sb[:])
    except _StopBuild:
        pass

    nc.compile()
    return nc


# ---------------------------------------------------------------------------
# Host-side input prep
# ---------------------------------------------------------------------------

def make_in_maps(inputs: dict, n_cores: int):
    """Per-core input dicts. Core 2b = up path of batch b, 2b+1 = down."""
    f32c = lambda a: np.ascontiguousarray(np.asarray(a, dtype=np.float32))

    def bake_mask(w):
        w = np.array(w, dtype=np.float32)
        w[-2:, :] = 1.0
        w[:, -2:] = 1.0
        return w
    maps = []
    for c in range(n_cores):
        b, down = divmod(c, 2)
        m = {
            "x": f32c(inputs["x"][b]),
            "p1": f32c(inputs["parameter1"]),
            "p2": f32c(inputs["parameter2"]),
        }
        if not down:
            m["adj"] = f32c(inputs["adj"][b])
            m["invT"] = f32c(inputs["up_inv_deg"][b].T)
            for l in range(3):
                m[f"w{l}a"] = bake_mask(inputs[f"l{l}_up_adj_w"])
                m[f"w{l}c"] = f32c(inputs[f"l{l}_up_w"])
                m[f"w{l}b"] = f32c(inputs[f"l{l}_bias"][:, :BH])
        else:
            m["adj"] = f32c(inputs["adj"][b].T)
            m["invT"] = f32c(inputs["down_inv_deg"][b].T)
            for l in range(3):
                m[f"w{l}a"] = bake_mask(inputs[f"l{l}_down_adj_w"].T)
                m[f"w{l}c"] = f32c(inputs[f"l{l}_down_w"])
                m[f"w{l}b"] = f32c(inputs[f"l{l}_bias"][:, BH:])
        maps.append(m)
    return maps


_nc_cache = {}


def _get_program(n_cores, N):
    key = (n_cores, N)
    if key not in _nc_cache:
        _nc_cache[key] = build_program(n_cores, N)
    return _nc_cache[key]


def kernel(**inputs) -> np.ndarray:
    n_cores = 8
    nc = _get_program(n_cores, N_FULL)
    in_maps = make_in_maps(inputs, n_cores)
    res = run_bass_kernel_spmd(nc, in_maps, core_ids=list(range(n_cores)))
    out = np.zeros((B, 1), dtype=np.float32)
    for b in range(B):
        out[b, 0] = res.results[2 * b]["ypred"][0, 0]
    return out



# revision 5
# speedup vs baseline: 2.3308x; 2.3308x over previous
"""BiGraphSAGEDecoder Trainium2 kernel (v2: fp16 + reassociated matmul chain).

Sharding: 8 cores = 4 batches x {up-path, down-path}. One SPMD bass program;
the up/down asymmetry is handled purely by data (down cores receive host-
transposed adjacency / adjacency-weight / inv-degree matrices).

Key restructurings vs the reference math:
  * associativity:  inv @ (prod^T @ h) @ W  ->  inv @ (prod^T @ (h @ W)),
    so both N x N matmuls run at width 256 instead of width din (768).
  * all feature maps are kept FEATURE-MAJOR (hT: [feature part, node free]);
    column L2 norms via a ones-vector matmul on the PE. No hidden-state
    transposes anywhere.
  * layer 2 computes only the two drug rows after a2 = prod^T @ g2
    (b restricted to 2 rows of inv).
  * fp16 storage end-to-end (PSUM accumulation fp32); host pre-converts.

Per layer (per core, its path), with prod = adj . Wadj' (mask baked on host):
  g    = h @ Wc                      (PE, lhsT = hT tiles)          [N, 256]
  a^T  = (prod^T @ g)^T              (PE, k-outer, 8 psum banks)    [256, N]
  a_sb = transpose(a^T)              (PE transposes)                [N, 256]
  b^T  = (inv @ a)^T                 (PE, lhsT = a_sb, rhs = invT)  [256, N]
  gb^T = (h @ Wb_half)^T             (PE, lhsT = Wb, rhs = hT)      [128, N]
  stage [384, N] = [b^T ; gb^T] -> 2-rank AllGather -> catT [768, N]
  hT'  = lrelu(catT / max(||catT||_col, eps))   (DVE + ones-matmul norms)
Head: bilinear form on PE in fp32 (tiny).
"""

import os
import sys
import types
import contextlib

sys.path.insert(0, "/opt/trn_rl_repo")

import numpy as np

import concourse.bass as bass
import concourse.tile as tile
from concourse import mybir, bacc
from concourse.mybir import AxisListType
from concourse.masks import make_identity
from concourse.bass_utils import run_bass_kernel_spmd

FP = mybir.dt.float32
F16 = mybir.dt.float16
BF = mybir.dt.bfloat16
AF = mybir.ActivationFunctionType
ALU = mybir.AluOpType

# ---------------------------------------------------------------------------
# Environment patches (required for this container's toolchain)
# ---------------------------------------------------------------------------


def install_ntff_shim():
    """antenv.axon_hooks is absent in this image; provide it so trace=True
    profiling works (used by test.py, harmless otherwise)."""
    try:
        import antenv.axon_hooks  # noqa: F401
        return
    except ImportError:
        pass
    try:
        import antenv
    except ImportError:
        return
    mod = types.ModuleType("antenv.axon_hooks")
    _holder = {"hook": None}
    mod.set_axon_ntff_profile_hook = lambda h: _holder.__setitem__("hook", h)
    mod.get_axon_ntff_profile_hook = lambda: _holder["hook"]
    sys.modules["antenv.axon_hooks"] = mod
    antenv.axon_hooks = mod
    try:
        from trn_agent_boot.trn_boot import _ntff_profile_via_ctypes

        hook = _ntff_profile_via_ctypes("/opt/axon/libaxon_pjrt.so")
        if hook is not None:
            mod.set_axon_ntff_profile_hook(hook)
    except Exception:
        pass


install_ntff_shim()

if os.environ.get("KGSD_LDW_OPT", "0") != "0":
    # let walrus dedup back-to-back LDWEIGHTS
    import concourse.bass_utils as _bu
    _orig_run_command = _bu.run_command

    def _patched_run_command(argv, **kw):
        argv = ["--enable-ldw-opt=true" if a == "--enable-ldw-opt=false"
                else a for a in argv]
        return _orig_run_command(argv, **kw)

    _bu.run_command = _patched_run_command

# ---------------------------------------------------------------------------
# Problem constants
# ---------------------------------------------------------------------------

N_FULL = 2048
B = 4
P = 128
DOUT = 256     # per-path cat chunk width
BW = 128       # bias half width per core
DEC = 128
DINS = (256, 768, 768)   # per-layer input dims
EPS = 1e-12
LEAK = 0.1


# ---------------------------------------------------------------------------
# Program builder
# ---------------------------------------------------------------------------

def build_program(n_cores: int, N: int = N_FULL, stop_phase: int = 99):
    NT = N // P                 # 128-row blocks
    CW = min(512, N)            # psum chunk width
    NI = N // CW                # chunks per full row
    NH = 2 if N >= 1024 else 1  # stage halves (for AG overlap)
    Nh = N // NH
    NCI = NI // NH              # chunks per half

    nc = bacc.Bacc("TRN2", target_bir_lowering=False, debug=False,
                   num_devices=n_cores)

    # --- DRAM I/O (all fp16 except head params / output) ---
    xT_d = nc.dram_tensor("xT", [DINS[0], N], F16, kind="ExternalInput")
    adj_d = nc.dram_tensor("adj", [N, N], F16, kind="ExternalInput")
    invT_d = nc.dram_tensor("invT", [N, N], F16, kind="ExternalInput")
    wa_d = [nc.dram_tensor(f"w{l}a", [N, N], F16, kind="ExternalInput")
            for l in range(3)]
    wc_d = [nc.dram_tensor(f"w{l}c", [DINS[l], DOUT], F16,
                           kind="ExternalInput") for l in range(3)]
    wb_d = [nc.dram_tensor(f"w{l}b", [DINS[l], BW], F16,
                           kind="ExternalInput") for l in range(3)]
    p1_d = nc.dram_tensor("p1", [3 * DOUT, DEC], FP, kind="ExternalInput")
    p2_d = nc.dram_tensor("p2", [DEC, DEC], FP, kind="ExternalInput")
    y_d = nc.dram_tensor("ypred", [1, 1], FP, kind="ExternalOutput")

    groups = [[i, i + 1] for i in range(0, n_cores, 2)]

    with tile.TileContext(nc) as tc:
        with contextlib.ExitStack() as ctx:
            const_p = ctx.enter_context(tc.tile_pool(name="const", bufs=1))
            adjc_p = ctx.enter_context(tc.tile_pool(name="adjc", bufs=1))
            h_p = ctx.enter_context(tc.tile_pool(name="h", bufs=1))
            g_p = ctx.enter_context(tc.tile_pool(name="g", bufs=1))
            a_p = ctx.enter_context(tc.tile_pool(name="a", bufs=1))
            aT_p = ctx.enter_context(tc.tile_pool(name="aT", bufs=1))
            stream_p = ctx.enter_context(tc.tile_pool(name="stream", bufs=3))
            w_p = ctx.enter_context(tc.tile_pool(name="w", bufs=1))
            sq_p = ctx.enter_context(tc.tile_pool(name="sq", bufs=2))
            rn_p = ctx.enter_context(tc.tile_pool(name="rn", bufs=2))
            misc_p = ctx.enter_context(tc.tile_pool(name="misc", bufs=2))
            psum_p = ctx.enter_context(
                tc.tile_pool(name="psum", bufs=8, space="PSUM"))
            dram_p = ctx.enter_context(
                tc.tile_pool(name="dram", bufs=2, space="DRAM"))

            ident16 = const_p.tile([P, P], F16, tag="id16")
            make_identity(nc, ident16)
            ident32 = const_p.tile([P, P], FP, tag="id32")
            make_identity(nc, ident32)
            ones16 = const_p.tile([P, 1], BF, tag="o16")
            nc.gpsimd.memset(ones16[:], 1.0)
            onesr = const_p.tile([1, P], FP, tag="o32")
            nc.gpsimd.memset(onesr[:], 1.0)

            # head params (tiny, load early)
            D3 = 3 * DOUT
            ND3 = D3 // P
            p1_t = const_p.tile([P, ND3 * DEC], FP, tag="p1")
            for d in range(ND3):
                nc.scalar.dma_start(p1_t[:, d * DEC:(d + 1) * DEC],
                                    p1_d.ap()[d * P:(d + 1) * P, :])
            p2_t = const_p.tile([P, DEC], FP, tag="p2")
            nc.scalar.dma_start(p2_t[:], p2_d.ap())

            # copy-engine alternation helper
            _alt = [0]

            def alt_copy(dst, src):
                if _alt[0] % 2 == 0:
                    nc.vector.tensor_copy(dst, src)
                else:
                    nc.scalar.copy(dst, src)
                _alt[0] += 1

            # l0 hT = xT (host-transposed input)
            def h_tiles(nd):
                return [h_p.tile([P, N], F16, tag=f"h{c}", name="hT")
                        for c in range(nd)]

            hT = h_tiles(DINS[0] // P)
            for d, t in enumerate(hT):
                nc.sync.dma_start(t[:], xT_d.ap()[d * P:(d + 1) * P, :])

            # persistent adj cache (fills during l0 a-phase consumption)
            adjc = [adjc_p.tile([P, N], F16, tag=f"adj{k}", name="adjc")
                    for k in range(NT)]
            for k in range(NT):
                nc.sync.dma_start(adjc[k][:], adj_d.ap()[k * P:(k + 1) * P, :])

            invTcols = misc_p.tile([P, NT * 2], F16, tag="ivc", bufs=1)
            dr = None

            for l in range(3):
                din = DINS[l]
                ND = din // P
                last = (l == 2)

                # ---- layer weights ----
                wc_t = w_p.tile([P, ND * DOUT], F16, tag="wc", name="wc_t")
                for d in range(ND):
                    nc.scalar.dma_start(wc_t[:, d * DOUT:(d + 1) * DOUT],
                                        wc_d[l].ap()[d * P:(d + 1) * P, :])
                wb_t = w_p.tile([P, ND * BW], F16, tag="wb", name="wb_t")
                for d in range(ND):
                    nc.scalar.dma_start(wb_t[:, d * BW:(d + 1) * BW],
                                        wb_d[l].ap()[d * P:(d + 1) * P, :])

                # ---- g = h @ Wc  [N, 256] node-major ----
                g_t = [g_p.tile([P, DOUT], F16, tag=f"g{it}", name="g_t")
                       for it in range(NT)]
                for it in range(NT):
                    pg = psum_p.tile([P, DOUT], FP, tag="ps", name="pg")
                    for d in range(ND):
                        nc.tensor.matmul(pg[:], hT[d][:, it * P:(it + 1) * P],
                                         wc_t[:, d * DOUT:(d + 1) * DOUT],
                                         start=(d == 0), stop=(d == ND - 1))
                    alt_copy(g_t[it][:], pg[:])

                if not last:
                    # ---- stage DRAM + gb^T = (h @ Wb_half)^T [128, N] ----
                    stage_h = [dram_p.tile([3 * P, Nh], F16, tag=f"stage{hh}",
                                           name="stage_h")
                               for hh in range(NH)]
                    pgb = [psum_p.tile([P, CW], FP, tag="ps", name="pgb")
                           for _ in range(NI)]
                    for d in range(ND):
                        for ic in range(NI):
                            nc.tensor.matmul(
                                pgb[ic][:], wb_t[:, d * BW:(d + 1) * BW],
                                hT[d][:, ic * CW:(ic + 1) * CW],
                                start=(d == 0), stop=(d == ND - 1))
                    for ic in range(NI):
                        stg = misc_p.tile([P, CW], F16, tag="stg", bufs=4,
                                          name="stg")
                        alt_copy(stg[:], pgb[ic][:])
                        hh, io = divmod(ic, NCI)
                        nc.scalar.dma_start(
                            stage_h[hh][2 * P:3 * P, io * CW:(io + 1) * CW],
                            stg[:])

                # ---- a-phase (k-outer): psum = a^T [256, N] ----
                pa = [[psum_p.tile([P, CW], FP, tag="ps", name="pa")
                       for _ in range(NI)] for _ in range(2)]
                for kt in range(NT):
                    wa_t = stream_p.tile([P, N], F16, tag="wa", name="wa_t")
                    nc.scalar.dma_start(wa_t[:],
                                        wa_d[l].ap()[kt * P:(kt + 1) * P, :])
                    prod = stream_p.tile([P, N], F16, tag="prod", name="prod")
                    nc.vector.tensor_tensor(prod[:], adjc[kt][:], wa_t[:],
                                            ALU.mult)
                    for mc in range(2):
                        lhsT = g_t[kt][:, mc * P:(mc + 1) * P]
                        for ic in range(NI):
                            nc.tensor.matmul(
                                pa[mc][ic][:], lhsT,
                                prod[:, ic * CW:(ic + 1) * CW],
                                start=(kt == 0), stop=(kt == NT - 1))

                # drain a^T then PE-transpose into a_sb [N, 256] node-major
                aT = [aT_p.tile([P, N], F16, tag=f"aT{mc}", name="aT")
                      for mc in range(2)]
                for mc in range(2):
                    for ic in range(NI):
                        alt_copy(aT[mc][:, ic * CW:(ic + 1) * CW],
                                 pa[mc][ic][:])
                a_sb = [a_p.tile([P, DOUT], F16, tag=f"a{jj}", name="a_sb")
                        for jj in range(NT)]
                for jj in range(NT):
                    for mc in range(2):
                        pt = psum_p.tile([P, P], F16, tag="ps", name="pt")
                        nc.tensor.transpose(pt[:],
                                            aT[mc][:, jj * P:(jj + 1) * P],
                                            ident16[:])
                        alt_copy(a_sb[jj][:, mc * P:(mc + 1) * P], pt[:])

                if not last:
                    # ---- b^T = (inv @ a)^T [256, N] ----
                    pb = [[psum_p.tile([P, CW], FP, tag="ps", name="pb")
                           for _ in range(NI)] for _ in range(2)]
                    for jj in range(NT):
                        iv = stream_p.tile([P, N], F16, tag="inv", name="iv")
                        nc.sync.dma_start(
                            iv[:], invT_d.ap()[jj * P:(jj + 1) * P, :])
                        if l == 1:
                            nc.vector.tensor_copy(
                                invTcols[:, jj * 2:(jj + 1) * 2],
                                iv[:, N - 2:N])
                        for cc in range(2):
                            lhsT = a_sb[jj][:, cc * P:(cc + 1) * P]
                            for ic in range(NI):
                                nc.tensor.matmul(
                                    pb[cc][ic][:], lhsT,
                                    iv[:, ic * CW:(ic + 1) * CW],
                                    start=(jj == 0), stop=(jj == NT - 1))
                    for cc in range(2):
                        for ic in range(NI):
                            stg = misc_p.tile([P, CW], F16, tag="stg", bufs=4,
                                              name="stg")
                            alt_copy(stg[:], pb[cc][ic][:])
                            hh, io = divmod(ic, NCI)
                            nc.scalar.dma_start(
                                stage_h[hh][cc * P:(cc + 1) * P,
                                            io * CW:(io + 1) * CW],
                                stg[:])

                    # ---- exchange (2-rank AllGather per half) ----
                    ag_h = []
                    for hh in range(NH):
                        agt = dram_p.tile([2, 3 * P, Nh], F16, tag=f"ag{hh}",
                                          name="ag_h")
                        nc.gpsimd.collective_compute(
                            "AllGather", ALU.bypass, replica_groups=groups,
                            ins=[stage_h[hh].opt()], outs=[agt.opt()])
                        ag_h.append(agt)

                    # ---- assemble catT [768, N] (= next hT), normalize ----
                    hT = h_tiles(6)
                    src = [(0, 0), (0, 1), (1, 0), (1, 1), (0, 2), (1, 2)]
                    for c, (r, rb) in enumerate(src):
                        for hh in range(NH):
                            nc.sync.dma_start(
                                hT[c][:, hh * Nh:(hh + 1) * Nh],
                                ag_h[hh][r, rb * P:(rb + 1) * P, :])
                    # col norms: ssq[1, i] = sum_c catT[c, i]^2 via ones-MM
                    pss = [psum_p.tile([1, CW], FP, tag="ps", name="pss")
                           for _ in range(NI)]
                    for c in range(6):
                        for hh in range(NH):
                            sq = sq_p.tile([P, Nh], BF, tag="sq", name="sq")
                            nc.vector.tensor_tensor(
                                sq[:], hT[c][:, hh * Nh:(hh + 1) * Nh],
                                hT[c][:, hh * Nh:(hh + 1) * Nh], ALU.mult)
                            for io in range(NCI):
                                ic = hh * NCI + io
                                nc.tensor.matmul(
                                    pss[ic][:], ones16[:],
                                    sq[:, io * CW:(io + 1) * CW],
                                    start=(c == 0), stop=(c == 5))
                    prn = []
                    for ic in range(NI):
                        rn = rn_p.tile([1, CW], FP, tag="rn", name="rn")
                        nc.scalar.activation(rn[:], pss[ic][:], AF.Sqrt)
                        nc.vector.tensor_scalar_max(rn[:], rn[:], EPS)
                        rr = rn_p.tile([1, CW], FP, tag="rr", name="rr")
                        nc.vector.reciprocal(rr[:], rn[:])
                        pr = psum_p.tile([P, CW], FP, tag="ps", name="pr")
                        nc.tensor.matmul(pr[:], onesr[0:1, :], rr[:],
                                         start=True, stop=True)
                        prn.append(pr)
                    for c in range(6):
                        for ic in range(NI):
                            sl = slice(ic * CW, (ic + 1) * CW)
                            nc.vector.tensor_tensor(hT[c][:, sl], hT[c][:, sl],
                                                    prn[ic][:], ALU.mult)
                            tmp = sq_p.tile([P, CW], F16, tag="lk", bufs=3,
                                            name="tmp")
                            nc.scalar.mul(tmp[:], hT[c][:, sl], LEAK)
                            nc.vector.tensor_max(hT[c][:, sl], hT[c][:, sl],
                                                 tmp[:])
                else:
                    # ---- l2: only drug rows N-2, N-1 ----
                    pb2 = psum_p.tile([2, DOUT], FP, tag="ps", name="pb2")
                    for jj in range(NT):
                        nc.tensor.matmul(pb2[:],
                                         invTcols[:, jj * 2:(jj + 1) * 2],
                                         a_sb[jj][:],
                                         start=(jj == 0), stop=(jj == NT - 1))
                    pbias = psum_p.tile([2, BW], FP, tag="ps", name="pbias")
                    for d in range(ND):
                        nc.tensor.matmul(pbias[:], hT[d][:, N - 2:N],
                                         wb_t[:, d * BW:(d + 1) * BW],
                                         start=(d == 0), stop=(d == ND - 1))
                    stage2 = dram_p.tile([2, 3 * P], F16, tag="st2", bufs=1,
                                         name="stage2")
                    s2 = misc_p.tile([2, 3 * P], F16, tag="s2", bufs=1,
                                     name="s2")
                    nc.vector.tensor_copy(s2[:, 0:DOUT], pb2[:])
                    nc.scalar.copy(s2[:, DOUT:3 * P], pbias[:])
                    nc.scalar.dma_start(stage2[:], s2[:])
                    ag2 = dram_p.tile([2, 2, 3 * P], F16, tag="ag2", bufs=1,
                                      name="ag2")
                    nc.gpsimd.collective_compute(
                        "AllGather", ALU.bypass, replica_groups=groups,
                        ins=[stage2.opt()], outs=[ag2.opt()])
                    drh = misc_p.tile([2, D3], F16, tag="drh", bufs=1,
                                      name="drh")
                    nc.sync.dma_start(drh[:, 0:DOUT], ag2[0, :, 0:DOUT])
                    nc.sync.dma_start(drh[:, DOUT:2 * DOUT],
                                      ag2[1, :, 0:DOUT])
                    nc.sync.dma_start(drh[:, 2 * DOUT:2 * DOUT + BW],
                                      ag2[0, :, DOUT:DOUT + BW])
                    nc.sync.dma_start(drh[:, 2 * DOUT + BW:D3],
                                      ag2[1, :, DOUT:DOUT + BW])
                    # normalize + leaky in fp32
                    dr = misc_p.tile([2, D3], FP, tag="dr", bufs=1, name="dr")
                    dsq = misc_p.tile([2, D3], FP, tag="dsq", bufs=1,
                                      name="dsq")
                    nc.vector.tensor_tensor(dsq[:], drh[:], drh[:], ALU.mult)
                    dssq = misc_p.tile([2, 1], FP, tag="dssq", bufs=1,
                                       name="dssq")
                    nc.vector.tensor_reduce(dssq[:], dsq[:], AxisListType.X,
                                            ALU.add)
                    dnrm = misc_p.tile([2, 1], FP, tag="dnrm", bufs=1,
                                       name="dnrm")
                    nc.scalar.activation(dnrm[:], dssq[:], AF.Sqrt)
                    nc.vector.tensor_scalar_max(dnrm[:], dnrm[:], EPS)
                    drn = misc_p.tile([2, 1], FP, tag="drn", bufs=1,
                                      name="drn")
                    nc.vector.reciprocal(drn[:], dnrm[:])
                    nc.vector.tensor_scalar(dr[:], drh[:], drn[:], None,
                                            ALU.mult)
                    nc.scalar.mul(dsq[:], dr[:], LEAK)
                    nc.vector.tensor_max(dr[:], dr[:], dsq[:])

            # ---- head: ypred = (a P1 P2) . (b P1)  (fp32, tiny) ----
            dT = misc_p.tile([P, ND3 * 2], FP, tag="dT", bufs=1)
            for d in range(ND3):
                pt = psum_p.tile([P, 2], FP, tag="ps", name="pth")
                nc.tensor.transpose(pt[:], dr[:, d * P:(d + 1) * P],
                                    ident32[0:2, 0:2])
                nc.vector.tensor_copy(dT[:, d * 2:(d + 1) * 2], pt[:])
            pw = psum_p.tile([P, 2], FP, tag="ps", name="pw")
            for d in range(ND3):
                nc.tensor.matmul(pw[:], p1_t[:, d * DEC:(d + 1) * DEC],
                                 dT[:, d * 2:(d + 1) * 2],
                                 start=(d == 0), stop=(d == ND3 - 1))
            w_sb = misc_p.tile([P, 2], FP, tag="w_sb", bufs=1)
            nc.vector.tensor_copy(w_sb[:], pw[:])
            ptt = psum_p.tile([P, 1], FP, tag="ps", name="ptt")
            nc.tensor.matmul(ptt[:], p2_t[:], w_sb[:, 0:1], start=True,
                             stop=True)
            t_sb = misc_p.tile([P, 1], FP, tag="t_sb", bufs=1)
            nc.vector.tensor_copy(t_sb[:], ptt[:])
            py = psum_p.tile([1, 1], FP, tag="ps", name="py")
            nc.tensor.matmul(py[:], t_sb[:], w_sb[:, 1:2], start=True,
                             stop=True)
            y_sb = misc_p.tile([1, 1], FP, tag="y_sb", bufs=1)
            nc.vector.tensor_copy(y_sb[:], py[:])
            nc.sync.dma_start(y_d.ap(), y_sb[:])

    nc.compile()
    return nc


# ---------------------------------------------------------------------------
# Host-side input prep
# ---------------------------------------------------------------------------

def make_in_maps(inputs: dict, n_cores: int):
    """Per-core input dicts. Core 2b = up path of batch b, 2b+1 = down."""
    f32 = lambda a: np.ascontiguousarray(np.asarray(a, dtype=np.float32))
    f16 = lambda a: np.ascontiguousarray(
        np.asarray(a, dtype=np.float32).astype(np.float16))

    def bake(w):
        w = np.array(w, dtype=np.float32, copy=True)
        w[-2:, :] = 1.0
        w[:, -2:] = 1.0
        return w

    maps = []
    for c in range(n_cores):
        b, down = divmod(c, 2)
        m = {
            "xT": f16(np.asarray(inputs["x"][b]).T),
            "p1": f32(inputs["parameter1"]),
            "p2": f32(inputs["parameter2"]),
        }
        if not down:
            m["adj"] = f16(inputs["adj"][b])
            m["invT"] = f16(np.asarray(inputs["up_inv_deg"][b]).T)
            for l in range(3):
                m[f"w{l}a"] = f16(bake(inputs[f"l{l}_up_adj_w"]))
                m[f"w{l}c"] = f16(inputs[f"l{l}_up_w"])
                m[f"w{l}b"] = f16(inputs[f"l{l}_bias"][:, :BW])
        else:
            m["adj"] = f16(np.asarray(inputs["adj"][b]).T)
            m["invT"] = f16(np.asarray(inputs["down_inv_deg"][b]).T)
            for l in range(3):
                m[f"w{l}a"] = f16(bake(inputs[f"l{l}_down_adj_w"]).T)
                m[f"w{l}c"] = f16(inputs[f"l{l}_down_w"])
                m[f"w{l}b"] = f16(inputs[f"l{l}_bias"][:, BW:])
        maps.append(m)
    return maps


_nc_cache = {}


def _get_program(n_cores, N):
    key = (n_cores, N)
    if key not in _nc_cache:
        _nc_cache[key] = build_program(n_cores, N)
    return _nc_cache[key]


def kernel(**inputs) -> np.ndarray:
    n_cores = 8
    nc = _get_program(n_cores, N_FULL)
    in_maps = make_in_maps(inputs, n_cores)
    res = run_bass_kernel_spmd(nc, in_maps, core_ids=list(range(n_cores)))
    out = np.zeros((B, 1), dtype=np.float32)
    for b in range(B):
        out[b, 0] = res.results[2 * b]["ypred"][0, 0]
    return out


# revision 13
# speedup vs baseline: 2.4556x; 1.0536x over previous
"""BiGraphSAGEDecoder Trainium2 kernel (v2: fp16 + reassociated matmul chain).

Sharding: 8 cores = 4 batches x {up-path, down-path}. One SPMD bass program;
the up/down asymmetry is handled purely by data (down cores receive host-
transposed adjacency / adjacency-weight / inv-degree matrices).

Key restructurings vs the reference math:
  * associativity:  inv @ (prod^T @ h) @ W  ->  inv @ (prod^T @ (h @ W)),
    so both N x N matmuls run at width 256 instead of width din (768).
  * all feature maps are kept FEATURE-MAJOR (hT: [feature part, node free]);
    column L2 norms via a ones-vector matmul on the PE. No hidden-state
    transposes anywhere.
  * layer 2 computes only the two drug rows after a2 = prod^T @ g2
    (b restricted to 2 rows of inv).
  * fp16 storage end-to-end (PSUM accumulation fp32); host pre-converts.

Per layer (per core, its path), with prod = adj . Wadj' (mask baked on host):
  g    = h @ Wc                      (PE, lhsT = hT tiles)          [N, 256]
  a^T  = (prod^T @ g)^T              (PE, k-outer, 8 psum banks)    [256, N]
  a_sb = transpose(a^T)              (PE transposes)                [N, 256]
  b^T  = (inv @ a)^T                 (PE, lhsT = a_sb, rhs = invT)  [256, N]
  gb^T = (h @ Wb_half)^T             (PE, lhsT = Wb, rhs = hT)      [128, N]
  stage [384, N] = [b^T ; gb^T] -> 2-rank AllGather -> catT [768, N]
  hT'  = lrelu(catT / max(||catT||_col, eps))   (DVE + ones-matmul norms)
Head: bilinear form on PE in fp32 (tiny).
"""

import os
import sys
import types
import contextlib

sys.path.insert(0, "/opt/trn_rl_repo")

import numpy as np

import concourse.bass as bass
import concourse.tile as tile
from concourse import mybir, bacc
from concourse.mybir import AxisListType
from concourse.masks import make_identity
from concourse.bass_utils import run_bass_kernel_spmd

FP = mybir.dt.float32
F16 = mybir.dt.float16
BF = mybir.dt.bfloat16
AF = mybir.ActivationFunctionType
ALU = mybir.AluOpType

# ---------------------------------------------------------------------------
# Environment patches (required for this container's toolchain)
# ---------------------------------------------------------------------------


def install_ntff_shim():
    """antenv.axon_hooks is absent in this image; provide it so trace=True
    profiling works (used by test.py, harmless otherwise)."""
    try:
        import antenv.axon_hooks  # noqa: F401
        return
    except ImportError:
        pass
    try:
        import antenv
    except ImportError:
        return
    mod = types.ModuleType("antenv.axon_hooks")
    _holder = {"hook": None}
    mod.set_axon_ntff_profile_hook = lambda h: _holder.__setitem__("hook", h)
    mod.get_axon_ntff_profile_hook = lambda: _holder["hook"]
    sys.modules["antenv.axon_hooks"] = mod
    antenv.axon_hooks = mod
    try:
        from trn_agent_boot.trn_boot import _ntff_profile_via_ctypes

        hook = _ntff_profile_via_ctypes("/opt/axon/libaxon_pjrt.so")
        if hook is not None:
            mod.set_axon_ntff_profile_hook(hook)
    except Exception:
        pass


install_ntff_shim()

if os.environ.get("KGSD_LDW_OPT", "0") != "0":
    # let walrus dedup back-to-back LDWEIGHTS
    import concourse.bass_utils as _bu
    _orig_run_command = _bu.run_command

    def _patched_run_command(argv, **kw):
        argv = ["--enable-ldw-opt=true" if a == "--enable-ldw-opt=false"
                else a for a in argv]
        return _orig_run_command(argv, **kw)

    _bu.run_command = _patched_run_command

# ---------------------------------------------------------------------------
# Problem constants
# ---------------------------------------------------------------------------

N_FULL = 2048
B = 4
P = 128
DOUT = 256     # per-path cat chunk width
BW = 128       # bias half width per core
DEC = 128
DINS = (256, 768, 768)   # per-layer input dims
EPS = 1e-12
LEAK = 0.1


# ---------------------------------------------------------------------------
# Program builder
# ---------------------------------------------------------------------------

def build_program(n_cores: int, N: int = N_FULL, stop_phase: int = 99):
    NT = N // P                 # 128-row blocks
    CW = min(512, N)            # psum chunk width
    NI = N // CW                # chunks per full row
    NH = 2 if N >= 1024 else 1  # stage halves (for AG overlap)
    Nh = N // NH
    NCI = NI // NH              # chunks per half

    nc = bacc.Bacc("TRN2", target_bir_lowering=False, debug=False,
                   num_devices=n_cores)

    # --- DRAM I/O (all fp16 except head params / output) ---
    xT_d = nc.dram_tensor("xT", [DINS[0], N], F16, kind="ExternalInput")
    adj_d = nc.dram_tensor("adj", [N, N], F16, kind="ExternalInput")
    invT_d = nc.dram_tensor("invT", [N, N], F16, kind="ExternalInput")
    wa_d = [nc.dram_tensor(f"w{l}a", [N, N], F16, kind="ExternalInput")
            for l in range(3)]
    wc_d = [nc.dram_tensor(f"w{l}c", [DINS[l], DOUT], F16,
                           kind="ExternalInput") for l in range(3)]
    # full bias weights (both halves) -- each core computes the whole bias
    # chunk locally so it never rides the AllGather
    wb_d = [nc.dram_tensor(f"w{l}b", [DINS[l], 2 * BW], F16,
                           kind="ExternalInput") for l in range(3)]
    p1_d = nc.dram_tensor("p1", [3 * DOUT, DEC], FP, kind="ExternalInput")
    p2_d = nc.dram_tensor("p2", [DEC, DEC], FP, kind="ExternalInput")
    y_d = nc.dram_tensor("ypred", [1, 1], FP, kind="ExternalOutput")

    groups = [[i, i + 1] for i in range(0, n_cores, 2)]

    with tile.TileContext(nc) as tc:
        with contextlib.ExitStack() as ctx:
            const_p = ctx.enter_context(tc.tile_pool(name="const", bufs=1))
            adjc_p = ctx.enter_context(tc.tile_pool(name="adjc", bufs=1))
            h_p = ctx.enter_context(tc.tile_pool(name="h", bufs=1))
            g_p = ctx.enter_context(tc.tile_pool(name="g", bufs=1))
            a_p = ctx.enter_context(tc.tile_pool(name="a", bufs=1))
            aT_p = ctx.enter_context(tc.tile_pool(name="aT", bufs=1))
            stream_p = ctx.enter_context(tc.tile_pool(name="stream", bufs=3))
            w_p = ctx.enter_context(tc.tile_pool(name="w", bufs=1))
            sq_p = ctx.enter_context(tc.tile_pool(name="sq", bufs=2))
            rn_p = ctx.enter_context(tc.tile_pool(name="rn", bufs=2))
            misc_p = ctx.enter_context(tc.tile_pool(name="misc", bufs=2))
            psum_p = ctx.enter_context(
                tc.tile_pool(name="psum", bufs=8, space="PSUM"))
            dram_p = ctx.enter_context(
                tc.tile_pool(name="dram", bufs=2, space="DRAM"))

            ident16 = const_p.tile([P, P], F16, tag="id16")
            make_identity(nc, ident16)
            ident32 = const_p.tile([P, P], FP, tag="id32")
            make_identity(nc, ident32)
            ones128 = const_p.tile([P, P], BF, tag="o128")
            nc.gpsimd.memset(ones128[:], 1.0)

            # head params (tiny, load early)
            D3 = 3 * DOUT
            ND3 = D3 // P
            p1_t = const_p.tile([P, ND3 * DEC], FP, tag="p1")
            for d in range(ND3):
                nc.scalar.dma_start(p1_t[:, d * DEC:(d + 1) * DEC],
                                    p1_d.ap()[d * P:(d + 1) * P, :])
            p2_t = const_p.tile([P, DEC], FP, tag="p2")
            nc.scalar.dma_start(p2_t[:], p2_d.ap())

            # copy-engine alternation helper
            _alt = [0]

            def alt_copy(dst, src):
                if _alt[0] % 2 == 0:
                    nc.vector.tensor_copy(dst, src)
                else:
                    nc.scalar.copy(dst, src)
                _alt[0] += 1

            # l0 hT = xT (host-transposed input)
            def h_tiles(nd):
                return [h_p.tile([P, N], F16, tag=f"h{c}", name="hT")
                        for c in range(nd)]

            hT = h_tiles(DINS[0] // P)
            for d, t in enumerate(hT):
                nc.sync.dma_start(t[:], xT_d.ap()[d * P:(d + 1) * P, :])

            # persistent adj cache (fills during l0 a-phase consumption)
            adjc = [adjc_p.tile([P, N], F16, tag=f"adj{k}", name="adjc")
                    for k in range(NT)]
            for k in range(NT):
                nc.sync.dma_start(adjc[k][:], adj_d.ap()[k * P:(k + 1) * P, :])

            invTcols = misc_p.tile([P, NT * 2], F16, tag="ivc", bufs=1)
            dr = None

            for l in range(3):
                din = DINS[l]
                ND = din // P
                last = (l == 2)

                # ---- layer weights ----
                wc_t = w_p.tile([P, ND * DOUT], F16, tag="wc", name="wc_t")
                for d in range(ND):
                    nc.scalar.dma_start(wc_t[:, d * DOUT:(d + 1) * DOUT],
                                        wc_d[l].ap()[d * P:(d + 1) * P, :])
                wb_t = w_p.tile([P, ND * 2 * BW], F16, tag="wb", name="wb_t")
                for d in range(ND):
                    nc.scalar.dma_start(
                        wb_t[:, d * 2 * BW:(d + 1) * 2 * BW],
                        wb_d[l].ap()[d * P:(d + 1) * P, :])

                # ---- g = h @ Wc  [N, 256] node-major ----
                g_t = [g_p.tile([P, DOUT], F16, tag=f"g{it}", name="g_t")
                       for it in range(NT)]
                for it in range(NT):
                    pg = psum_p.tile([P, DOUT], FP, tag="ps", name="pg")
                    for d in range(ND):
                        nc.tensor.matmul(pg[:], hT[d][:, it * P:(it + 1) * P],
                                         wc_t[:, d * DOUT:(d + 1) * DOUT],
                                         start=(d == 0), stop=(d == ND - 1))
                    alt_copy(g_t[it][:], pg[:])

                if not last:
                    # ---- gb^T = (h @ Wb_full)^T [256, N] -> next-h c4/c5
                    # directly (bias chunk is replicated work on both cores
                    # of the pair; it skips the AllGather entirely) ----
                    hT_next = h_tiles(6)
                    pgb = [[psum_p.tile([P, CW], FP, tag="ps", name="pgb")
                            for _ in range(NI)] for _ in range(2)]
                    for d in range(ND):
                        for cc in range(2):
                            lhsT = wb_t[:, d * 2 * BW + cc * BW:
                                        d * 2 * BW + (cc + 1) * BW]
                            for ic in range(NI):
                                nc.tensor.matmul(
                                    pgb[cc][ic][:], lhsT,
                                    hT[d][:, ic * CW:(ic + 1) * CW],
                                    start=(d == 0), stop=(d == ND - 1))
                    for cc in range(2):
                        for ic in range(NI):
                            alt_copy(
                                hT_next[4 + cc][:, ic * CW:(ic + 1) * CW],
                                pgb[cc][ic][:])

                # ---- a-phase (k-outer): psum = a^T [256, N] ----
                pa = [[psum_p.tile([P, CW], FP, tag="ps", name="pa")
                       for _ in range(NI)] for _ in range(2)]
                for kt in range(NT):
                    wa_t = stream_p.tile([P, N], F16, tag="wa", name="wa_t")
                    nc.scalar.dma_start(wa_t[:],
                                        wa_d[l].ap()[kt * P:(kt + 1) * P, :])
                    prod = stream_p.tile([P, N], F16, tag="prod", name="prod")
                    nc.vector.tensor_tensor(prod[:], adjc[kt][:], wa_t[:],
                                            ALU.mult)
                    for mc in range(2):
                        lhsT = g_t[kt][:, mc * P:(mc + 1) * P]
                        for ic in range(NI):
                            nc.tensor.matmul(
                                pa[mc][ic][:], lhsT,
                                prod[:, ic * CW:(ic + 1) * CW],
                                start=(kt == 0), stop=(kt == NT - 1))

                # drain a^T then PE-transpose into a_sb [N, 256] node-major
                aT = [aT_p.tile([P, N], F16, tag=f"aT{mc}", name="aT")
                      for mc in range(2)]
                for mc in range(2):
                    for ic in range(NI):
                        alt_copy(aT[mc][:, ic * CW:(ic + 1) * CW],
                                 pa[mc][ic][:])
                a_sb = [a_p.tile([P, DOUT], F16, tag=f"a{jj}", name="a_sb")
                        for jj in range(NT)]
                for jj in range(NT):
                    for mc in range(2):
                        pt = psum_p.tile([P, P], F16, tag="ps", name="pt")
                        nc.tensor.transpose(pt[:],
                                            aT[mc][:, jj * P:(jj + 1) * P],
                                            ident16[:])
                        alt_copy(a_sb[jj][:, mc * P:(mc + 1) * P], pt[:])

                if not last:
                    # ---- b^T = (inv @ a)^T [256, N] ----
                    pb = [[psum_p.tile([P, CW], FP, tag="ps", name="pb")
                           for _ in range(NI)] for _ in range(2)]
                    for jj in range(NT):
                        iv = stream_p.tile([P, N], F16, tag="inv", name="iv")
                        nc.sync.dma_start(
                            iv[:], invT_d.ap()[jj * P:(jj + 1) * P, :])
                        if l == 1:
                            nc.vector.tensor_copy(
                                invTcols[:, jj * 2:(jj + 1) * 2],
                                iv[:, N - 2:N])
                        for cc in range(2):
                            lhsT = a_sb[jj][:, cc * P:(cc + 1) * P]
                            for ic in range(NI):
                                nc.tensor.matmul(
                                    pb[cc][ic][:], lhsT,
                                    iv[:, ic * CW:(ic + 1) * CW],
                                    start=(jj == 0), stop=(jj == NT - 1))
                    # drain b^T into per-quarter DRAM stages (fine-grained so
                    # each quarter's AllGather + readback + norm pipelines
                    # against the next layer's g-phase)
                    stage_q = [dram_p.tile([2 * P, CW], F16, tag=f"stq{ic}",
                                           name="stage_q")
                               for ic in range(NI)]
                    for cc in range(2):
                        for ic in range(NI):
                            stg = misc_p.tile([P, CW], F16, tag="stg", bufs=4,
                                              name="stg")
                            alt_copy(stg[:], pb[cc][ic][:])
                            nc.scalar.dma_start(
                                stage_q[ic][cc * P:(cc + 1) * P, :], stg[:])

                    # ---- exchange (2-rank AllGather per quarter) ----
                    ag_q = []
                    for ic in range(NI):
                        agt = dram_p.tile([2, 2 * P, CW], F16, tag=f"agq{ic}",
                                          name="ag_q")
                        nc.gpsimd.collective_compute(
                            "AllGather", ALU.bypass, replica_groups=groups,
                            ins=[stage_q[ic].opt()], outs=[agt.opt()])
                        ag_q.append(agt)

                    # ---- assemble catT [768, N] (= next hT), normalize ----
                    src = [(0, 0), (0, 1), (1, 0), (1, 1)]
                    for ic in range(NI):
                        for c, (r, rb) in enumerate(src):
                            nc.sync.dma_start(
                                hT_next[c][:, ic * CW:(ic + 1) * CW],
                                ag_q[ic][r, rb * P:(rb + 1) * P, :])
                    # col norms, quarter at a time: ssq replicated onto all
                    # 128 partitions via ones-matrix matmul, then wide
                    # max/sqrt/recip (no serial 1-partition ops)
                    for ic in range(NI):
                        sl = slice(ic * CW, (ic + 1) * CW)
                        psw = psum_p.tile([P, CW], FP, tag="ps", name="psw")
                        for c in range(6):
                            sq = sq_p.tile([P, CW], BF, tag="sq", name="sq")
                            nc.vector.tensor_tensor(
                                sq[:], hT_next[c][:, sl], hT_next[c][:, sl],
                                ALU.mult)
                            nc.tensor.matmul(psw[:], ones128[:], sq[:],
                                             start=(c == 0), stop=(c == 5))
                        nrm = rn_p.tile([P, CW], FP, tag="nrm", name="nrm")
                        nc.vector.tensor_scalar_max(nrm[:], psw[:],
                                                    EPS * EPS)
                        nc.scalar.activation(nrm[:], nrm[:], AF.Sqrt)
                        rn = rn_p.tile([P, CW], FP, tag="rn", name="rn")
                        nc.vector.reciprocal(rn[:], nrm[:])
                        for c in range(6):
                            nc.vector.tensor_tensor(hT_next[c][:, sl],
                                                    hT_next[c][:, sl],
                                                    rn[:], ALU.mult)
                            tmp = sq_p.tile([P, CW], F16, tag="lk", bufs=3,
                                            name="tmp")
                            nc.scalar.mul(tmp[:], hT_next[c][:, sl], LEAK)
                            nc.vector.tensor_max(hT_next[c][:, sl],
                                                 hT_next[c][:, sl], tmp[:])
                    hT = hT_next
                else:
                    # ---- l2: only drug rows N-2, N-1 ----
                    pb2 = psum_p.tile([2, DOUT], FP, tag="ps", name="pb2")
                    for jj in range(NT):
                        nc.tensor.matmul(pb2[:],
                                         invTcols[:, jj * 2:(jj + 1) * 2],
                                         a_sb[jj][:],
                                         start=(jj == 0), stop=(jj == NT - 1))
                    # full bias chunk for the two drug rows (local, no AG)
                    pbias = psum_p.tile([2, 2 * BW], FP, tag="ps",
                                        name="pbias")
                    for d in range(ND):
                        nc.tensor.matmul(
                            pbias[:], hT[d][:, N - 2:N],
                            wb_t[:, d * 2 * BW:(d + 1) * 2 * BW],
                            start=(d == 0), stop=(d == ND - 1))
                    stage2 = dram_p.tile([2, DOUT], F16, tag="stl2", bufs=1,
                                         name="stage2")
                    s2 = misc_p.tile([2, DOUT], F16, tag="s2", bufs=1,
                                     name="s2")
                    nc.vector.tensor_copy(s2[:], pb2[:])
                    nc.scalar.dma_start(stage2[:], s2[:])
                    ag2 = dram_p.tile([2, 2, DOUT], F16, tag="ag2", bufs=1,
                                      name="ag2")
                    nc.gpsimd.collective_compute(
                        "AllGather", ALU.bypass, replica_groups=groups,
                        ins=[stage2.opt()], outs=[ag2.opt()])
                    drh = misc_p.tile([2, D3], F16, tag="drh", bufs=1,
                                      name="drh")
                    nc.sync.dma_start(drh[:, 0:DOUT], ag2[0, :, :])
                    nc.sync.dma_start(drh[:, DOUT:2 * DOUT], ag2[1, :, :])
                    nc.scalar.copy(drh[:, 2 * DOUT:D3], pbias[:])
                    # normalize + leaky in fp32
                    dr = misc_p.tile([2, D3], FP, tag="dr", bufs=1, name="dr")
                    dsq = misc_p.tile([2, D3], FP, tag="dsq", bufs=1,
                                      name="dsq")
                    nc.vector.tensor_tensor(dsq[:], drh[:], drh[:], ALU.mult)
                    dssq = misc_p.tile([2, 1], FP, tag="dssq", bufs=1,
                                       name="dssq")
                    nc.vector.tensor_reduce(dssq[:], dsq[:], AxisListType.X,
                                            ALU.add)
                    dnrm = misc_p.tile([2, 1], FP, tag="dnrm", bufs=1,
                                       name="dnrm")
                    nc.scalar.activation(dnrm[:], dssq[:], AF.Sqrt)
                    nc.vector.tensor_scalar_max(dnrm[:], dnrm[:], EPS)
                    drn = misc_p.tile([2, 1], FP, tag="drn", bufs=1,
                                      name="drn")
                    nc.vector.reciprocal(drn[:], dnrm[:])
                    nc.vector.tensor_scalar(dr[:], drh[:], drn[:], None,
                                            ALU.mult)
                    nc.scalar.mul(dsq[:], dr[:], LEAK)
                    nc.vector.tensor_max(dr[:], dr[:], dsq[:])

            # ---- head: ypred = (a P1 P2) . (b P1)  (fp32, tiny) ----
            dT = misc_p.tile([P, ND3 * 2], FP, tag="dT", bufs=1)
            for d in range(ND3):
                pt = psum_p.tile([P, 2], FP, tag="ps", name="pth")
                nc.tensor.transpose(pt[:], dr[:, d * P:(d + 1) * P],
                                    ident32[0:2, 0:2])
                nc.vector.tensor_copy(dT[:, d * 2:(d + 1) * 2], pt[:])
            pw = psum_p.tile([P, 2], FP, tag="ps", name="pw")
            for d in range(ND3):
                nc.tensor.matmul(pw[:], p1_t[:, d * DEC:(d + 1) * DEC],
                                 dT[:, d * 2:(d + 1) * 2],
                                 start=(d == 0), stop=(d == ND3 - 1))
            w_sb = misc_p.tile([P, 2], FP, tag="w_sb", bufs=1)
            nc.vector.tensor_copy(w_sb[:], pw[:])
            ptt = psum_p.tile([P, 1], FP, tag="ps", name="ptt")
            nc.tensor.matmul(ptt[:], p2_t[:], w_sb[:, 0:1], start=True,
                             stop=True)
            t_sb = misc_p.tile([P, 1], FP, tag="t_sb", bufs=1)
            nc.vector.tensor_copy(t_sb[:], ptt[:])
            py = psum_p.tile([1, 1], FP, tag="ps", name="py")
            nc.tensor.matmul(py[:], t_sb[:], w_sb[:, 1:2], start=True,
                             stop=True)
            y_sb = misc_p.tile([1, 1], FP, tag="y_sb", bufs=1)
            nc.vector.tensor_copy(y_sb[:], py[:])
            nc.sync.dma_start(y_d.ap(), y_sb[:])

    nc.compile()
    return nc


# ---------------------------------------------------------------------------
# Host-side input prep
# ---------------------------------------------------------------------------

def make_in_maps(inputs: dict, n_cores: int):
    """Per-core input dicts. Core 2b = up path of batch b, 2b+1 = down."""
    f32 = lambda a: np.ascontiguousarray(np.asarray(a, dtype=np.float32))
    f16 = lambda a: np.ascontiguousarray(
        np.asarray(a, dtype=np.float32).astype(np.float16))

    def bake(w):
        w = np.array(w, dtype=np.float32, copy=True)
        w[-2:, :] = 1.0
        w[:, -2:] = 1.0
        return w

    maps = []
    for c in range(n_cores):
        b, down = divmod(c, 2)
        m = {
            "xT": f16(np.asarray(inputs["x"][b]).T),
            "p1": f32(inputs["parameter1"]),
            "p2": f32(inputs["parameter2"]),
        }
        for l in range(3):
            m[f"w{l}b"] = f16(inputs[f"l{l}_bias"])  # full, both cores
        if not down:
            m["adj"] = f16(inputs["adj"][b])
            m["invT"] = f16(np.asarray(inputs["up_inv_deg"][b]).T)
            for l in range(3):
                m[f"w{l}a"] = f16(bake(inputs[f"l{l}_up_adj_w"]))
                m[f"w{l}c"] = f16(inputs[f"l{l}_up_w"])
        else:
            m["adj"] = f16(np.asarray(inputs["adj"][b]).T)
            m["invT"] = f16(np.asarray(inputs["down_inv_deg"][b]).T)
            for l in range(3):
                m[f"w{l}a"] = f16(bake(inputs[f"l{l}_down_adj_w"]).T)
                m[f"w{l}c"] = f16(inputs[f"l{l}_down_w"])
        maps.append(m)
    return maps


_nc_cache = {}


def _get_program(n_cores, N):
    key = (n_cores, N)
    if key not in _nc_cache:
        _nc_cache[key] = build_program(n_cores, N)
    return _nc_cache[key]


def kernel(**inputs) -> np.ndarray:
    n_cores = 8
    nc = _get_program(n_cores, N_FULL)
    in_maps = make_in_maps(inputs, n_cores)
    res = run_bass_kernel_spmd(nc, in_maps, core_ids=list(range(n_cores)))
    out = np.zeros((B, 1), dtype=np.float32)
    for b in range(B):
        out[b, 0] = res.results[2 * b]["ypred"][0, 0]
    return out


# revision 18
# speedup vs baseline: 2.6882x; 1.0947x over previous
"""BiGraphSAGEDecoder Trainium2 kernel (v2: fp16 + reassociated matmul chain).

Sharding: 8 cores = 4 batches x {up-path, down-path}. One SPMD bass program;
the up/down asymmetry is handled purely by data (down cores receive host-
transposed adjacency / adjacency-weight / inv-degree matrices).

Key restructurings vs the reference math:
  * associativity:  inv @ (prod^T @ h) @ W  ->  inv @ (prod^T @ (h @ W)),
    so both N x N matmuls run at width 256 instead of width din (768).
  * all feature maps are kept FEATURE-MAJOR (hT: [feature part, node free]);
    column L2 norms via a ones-vector matmul on the PE. No hidden-state
    transposes anywhere.
  * layer 2 computes only the two drug rows after a2 = prod^T @ g2
    (b restricted to 2 rows of inv).
  * fp16 storage end-to-end (PSUM accumulation fp32); host pre-converts.

Per layer (per core, its path), with prod = adj . Wadj' (mask baked on host):
  g    = h @ Wc                      (PE, lhsT = hT tiles)          [N, 256]
  a^T  = (prod^T @ g)^T              (PE, k-outer, 8 psum banks)    [256, N]
  a_sb = transpose(a^T)              (PE transposes)                [N, 256]
  b^T  = (inv @ a)^T                 (PE, lhsT = a_sb, rhs = invT)  [256, N]
  gb^T = (h @ Wb_half)^T             (PE, lhsT = Wb, rhs = hT)      [128, N]
  stage [384, N] = [b^T ; gb^T] -> 2-rank AllGather -> catT [768, N]
  hT'  = lrelu(catT / max(||catT||_col, eps))   (DVE + ones-matmul norms)
Head: bilinear form on PE in fp32 (tiny).
"""

import os
import sys
import types
import contextlib

sys.path.insert(0, "/opt/trn_rl_repo")

import numpy as np

import concourse.bass as bass
import concourse.tile as tile
from concourse import mybir, bacc
from concourse.mybir import AxisListType
from concourse.masks import make_identity
from concourse.bass_utils import run_bass_kernel_spmd

FP = mybir.dt.float32
F16 = mybir.dt.float16
BF = mybir.dt.bfloat16
AF = mybir.ActivationFunctionType
ALU = mybir.AluOpType

# ---------------------------------------------------------------------------
# Environment patches (required for this container's toolchain)
# ---------------------------------------------------------------------------


def install_ntff_shim():
    """antenv.axon_hooks is absent in this image; provide it so trace=True
    profiling works (used by test.py, harmless otherwise)."""
    try:
        import antenv.axon_hooks  # noqa: F401
        return
    except ImportError:
        pass
    try:
        import antenv
    except ImportError:
        return
    mod = types.ModuleType("antenv.axon_hooks")
    _holder = {"hook": None}
    mod.set_axon_ntff_profile_hook = lambda h: _holder.__setitem__("hook", h)
    mod.get_axon_ntff_profile_hook = lambda: _holder["hook"]
    sys.modules["antenv.axon_hooks"] = mod
    antenv.axon_hooks = mod
    try:
        from trn_agent_boot.trn_boot import _ntff_profile_via_ctypes

        hook = _ntff_profile_via_ctypes("/opt/axon/libaxon_pjrt.so")
        if hook is not None:
            mod.set_axon_ntff_profile_hook(hook)
    except Exception:
        pass


install_ntff_shim()

if os.environ.get("KGSD_LDW_OPT", "0") != "0":
    # let walrus dedup back-to-back LDWEIGHTS
    import concourse.bass_utils as _bu
    _orig_run_command = _bu.run_command

    def _patched_run_command(argv, **kw):
        argv = ["--enable-ldw-opt=true" if a == "--enable-ldw-opt=false"
                else a for a in argv]
        return _orig_run_command(argv, **kw)

    _bu.run_command = _patched_run_command

# ---------------------------------------------------------------------------
# Problem constants
# ---------------------------------------------------------------------------

N_FULL = 2048
B = 4
P = 128
DOUT = 256     # per-path cat chunk width
BW = 128       # bias half width per core
DEC = 128
DINS = (256, 768, 768)   # per-layer input dims
EPS = 1e-12
LEAK = 0.1


# ---------------------------------------------------------------------------
# Program builder
# ---------------------------------------------------------------------------

def build_program(n_cores: int, N: int = N_FULL, stop_phase: int = 99):
    NT = N // P                 # 128-row blocks
    CW = min(512, N)            # psum chunk width
    NI = N // CW                # chunks per full row
    NH = 2 if N >= 1024 else 1  # stage halves (for AG overlap)
    Nh = N // NH
    NCI = NI // NH              # chunks per half

    nc = bacc.Bacc("TRN2", target_bir_lowering=False, debug=False,
                   num_devices=n_cores)

    # --- DRAM I/O (all fp16 except head params / output) ---
    xT_d = nc.dram_tensor("xT", [DINS[0], N], F16, kind="ExternalInput")
    adj_d = nc.dram_tensor("adj", [N, N], F16, kind="ExternalInput")
    invT_d = nc.dram_tensor("invT", [N, N], F16, kind="ExternalInput")
    wa_d = [nc.dram_tensor(f"w{l}a", [N, N], F16, kind="ExternalInput")
            for l in range(3)]
    wc_d = [nc.dram_tensor(f"w{l}c", [DINS[l], DOUT], F16,
                           kind="ExternalInput") for l in range(3)]
    # full bias weights (both halves) -- each core computes the whole bias
    # chunk locally so it never rides the AllGather
    wb_d = [nc.dram_tensor(f"w{l}b", [DINS[l], 2 * BW], F16,
                           kind="ExternalInput") for l in range(3)]
    p1_d = nc.dram_tensor("p1", [3 * DOUT, DEC], FP, kind="ExternalInput")
    p2_d = nc.dram_tensor("p2", [DEC, DEC], FP, kind="ExternalInput")
    y_d = nc.dram_tensor("ypred", [1, 1], FP, kind="ExternalOutput")

    groups = [[i, i + 1] for i in range(0, n_cores, 2)]

    with tile.TileContext(nc) as tc:
        with contextlib.ExitStack() as ctx:
            const_p = ctx.enter_context(tc.tile_pool(name="const", bufs=1))
            adjc_p = ctx.enter_context(tc.tile_pool(name="adjc", bufs=1))
            h_p = ctx.enter_context(tc.tile_pool(name="h", bufs=2))
            g_p = ctx.enter_context(tc.tile_pool(name="g", bufs=1))
            a_p = ctx.enter_context(tc.tile_pool(name="a", bufs=1))
            aT_p = ctx.enter_context(tc.tile_pool(name="aT", bufs=1))
            stream_p = ctx.enter_context(tc.tile_pool(name="stream", bufs=3))
            w_p = ctx.enter_context(tc.tile_pool(name="w", bufs=1))
            sq_p = ctx.enter_context(tc.tile_pool(name="sq", bufs=2))
            rn_p = ctx.enter_context(tc.tile_pool(name="rn", bufs=2))
            misc_p = ctx.enter_context(tc.tile_pool(name="misc", bufs=2))
            psum_p = ctx.enter_context(
                tc.tile_pool(name="psum", bufs=8, space="PSUM"))
            dram_p = ctx.enter_context(
                tc.tile_pool(name="dram", bufs=2, space="DRAM"))

            ident16 = const_p.tile([P, P], F16, tag="id16")
            make_identity(nc, ident16)
            ident32 = const_p.tile([P, P], FP, tag="id32")
            make_identity(nc, ident32)
            ones128 = const_p.tile([P, P], BF, tag="o128")
            nc.gpsimd.memset(ones128[:], 1.0)

            # head params (tiny, load early)
            D3 = 3 * DOUT
            ND3 = D3 // P
            p1_t = const_p.tile([P, ND3 * DEC], FP, tag="p1")
            for d in range(ND3):
                nc.scalar.dma_start(p1_t[:, d * DEC:(d + 1) * DEC],
                                    p1_d.ap()[d * P:(d + 1) * P, :])
            p2_t = const_p.tile([P, DEC], FP, tag="p2")
            nc.scalar.dma_start(p2_t[:], p2_d.ap())

            # copy-engine alternation helper
            _alt = [0]

            def alt_copy(dst, src):
                if _alt[0] % 2 == 0:
                    nc.vector.tensor_copy(dst, src)
                else:
                    nc.scalar.copy(dst, src)
                _alt[0] += 1

            # l0 hT = xT (host-transposed input)
            def h_tiles(nd):
                return [h_p.tile([P, N], F16, tag=f"h{c}", name="hT")
                        for c in range(nd)]

            hT = h_tiles(DINS[0] // P)
            for d, t in enumerate(hT):
                nc.sync.dma_start(t[:], xT_d.ap()[d * P:(d + 1) * P, :])

            # persistent adj cache (fills during l0 a-phase consumption)
            adjc = [adjc_p.tile([P, N], F16, tag=f"adj{k}", name="adjc")
                    for k in range(NT)]
            for k in range(NT):
                nc.sync.dma_start(adjc[k][:], adj_d.ap()[k * P:(k + 1) * P, :])

            invTcols = misc_p.tile([P, NT * 2], F16, tag="ivc", bufs=1)
            dr = None

            for l in range(3):
                din = DINS[l]
                ND = din // P
                last = (l == 2)

                # ---- layer weights ----
                wc_t = w_p.tile([P, ND * DOUT], F16, tag="wc", name="wc_t")
                for d in range(ND):
                    nc.scalar.dma_start(wc_t[:, d * DOUT:(d + 1) * DOUT],
                                        wc_d[l].ap()[d * P:(d + 1) * P, :])
                wb_t = w_p.tile([P, ND * 2 * BW], F16, tag="wb", name="wb_t")
                for d in range(ND):
                    nc.scalar.dma_start(
                        wb_t[:, d * 2 * BW:(d + 1) * 2 * BW],
                        wb_d[l].ap()[d * P:(d + 1) * P, :])

                # ---- g = h @ Wc  [N, 256] node-major ----
                g_t = [g_p.tile([P, DOUT], F16, tag=f"g{it}", name="g_t")
                       for it in range(NT)]
                for it in range(NT):
                    pg = psum_p.tile([P, DOUT], FP, tag="ps", name="pg")
                    for d in range(ND):
                        nc.tensor.matmul(pg[:], hT[d][:, it * P:(it + 1) * P],
                                         wc_t[:, d * DOUT:(d + 1) * DOUT],
                                         start=(d == 0), stop=(d == ND - 1))
                    alt_copy(g_t[it][:], pg[:])

                if not last:
                    hT_next = h_tiles(6)

                # ---- a-phase (k-outer): psum = a^T [256, N] ----
                pa = [[psum_p.tile([P, CW], FP, tag="ps", name="pa")
                       for _ in range(NI)] for _ in range(2)]
                for kt in range(NT):
                    wa_t = stream_p.tile([P, N], F16, tag="wa", name="wa_t")
                    nc.scalar.dma_start(wa_t[:],
                                        wa_d[l].ap()[kt * P:(kt + 1) * P, :])
                    prod = stream_p.tile([P, N], F16, tag="prod", name="prod")
                    nc.vector.tensor_tensor(prod[:], adjc[kt][:], wa_t[:],
                                            ALU.mult)
                    for mc in range(2):
                        lhsT = g_t[kt][:, mc * P:(mc + 1) * P]
                        for ic in range(NI):
                            nc.tensor.matmul(
                                pa[mc][ic][:], lhsT,
                                prod[:, ic * CW:(ic + 1) * CW],
                                start=(kt == 0), stop=(kt == NT - 1))

                # drain a^T then PE-transpose into a_sb [N, 256] node-major
                aT = [aT_p.tile([P, N], F16, tag=f"aT{mc}", name="aT")
                      for mc in range(2)]
                for mc in range(2):
                    for ic in range(NI):
                        alt_copy(aT[mc][:, ic * CW:(ic + 1) * CW],
                                 pa[mc][ic][:])
                a_sb = [a_p.tile([P, DOUT], F16, tag=f"a{jj}", name="a_sb")
                        for jj in range(NT)]
                for jj in range(NT):
                    for mc in range(2):
                        pt = psum_p.tile([P, P], F16, tag="ps", name="pt")
                        nc.tensor.transpose(pt[:],
                                            aT[mc][:, jj * P:(jj + 1) * P],
                                            ident16[:])
                        alt_copy(a_sb[jj][:, mc * P:(mc + 1) * P], pt[:])

                if not last:
                    # ---- b^T = (inv @ a)^T [256, N], in NH half-width
                    # waves: wave hh streams invT cols [hh*Nh, (hh+1)*Nh)
                    # so wave 0's AllGather hides behind wave 1's matmuls ----
                    ag_q = []
                    for hh in range(NH):
                        pb = [[psum_p.tile([P, CW], FP, tag="ps", name="pb")
                               for _ in range(NCI)] for _ in range(2)]
                        for jj in range(NT):
                            iv = stream_p.tile([P, Nh], F16, tag="inv",
                                               name="iv")
                            nc.sync.dma_start(
                                iv[:], invT_d.ap()[jj * P:(jj + 1) * P,
                                                   hh * Nh:(hh + 1) * Nh])
                            if l == 1 and hh == NH - 1:
                                nc.vector.tensor_copy(
                                    invTcols[:, jj * 2:(jj + 1) * 2],
                                    iv[:, Nh - 2:Nh])
                            for cc in range(2):
                                lhsT = a_sb[jj][:, cc * P:(cc + 1) * P]
                                for io in range(NCI):
                                    nc.tensor.matmul(
                                        pb[cc][io][:], lhsT,
                                        iv[:, io * CW:(io + 1) * CW],
                                        start=(jj == 0), stop=(jj == NT - 1))
                        stage_q = dram_p.tile([2 * P, Nh], F16,
                                              tag=f"stq{hh}", name="stage_q")
                        for cc in range(2):
                            for io in range(NCI):
                                stg = misc_p.tile([P, CW], F16, tag="stg",
                                                  bufs=4, name="stg")
                                alt_copy(stg[:], pb[cc][io][:])
                                nc.scalar.dma_start(
                                    stage_q[cc * P:(cc + 1) * P,
                                            io * CW:(io + 1) * CW], stg[:])
                        agt = dram_p.tile([2, 2 * P, Nh], F16, tag=f"agq{hh}",
                                          name="ag_q")
                        nc.gpsimd.collective_compute(
                            "AllGather", ALU.bypass, replica_groups=groups,
                            ins=[stage_q.opt()], outs=[agt.opt()])
                        ag_q.append(agt)

                    # ---- gb^T = (h @ Wb_full)^T [256, N] -> c4/c5 locally;
                    # runs on the PE during the AllGather flight ----
                    pgb = [[psum_p.tile([P, CW], FP, tag="ps", name="pgb")
                            for _ in range(NI)] for _ in range(2)]
                    for d in range(ND):
                        for cc in range(2):
                            lhsT = wb_t[:, d * 2 * BW + cc * BW:
                                        d * 2 * BW + (cc + 1) * BW]
                            for ic in range(NI):
                                nc.tensor.matmul(
                                    pgb[cc][ic][:], lhsT,
                                    hT[d][:, ic * CW:(ic + 1) * CW],
                                    start=(d == 0), stop=(d == ND - 1))
                    for cc in range(2):
                        for ic in range(NI):
                            alt_copy(
                                hT_next[4 + cc][:, ic * CW:(ic + 1) * CW],
                                pgb[cc][ic][:])

                    # ---- assemble catT [768, N] (= next hT), normalize ----
                    src = [(0, 0), (0, 1), (1, 0), (1, 1)]
                    for hh in range(NH):
                        for c, (r, rb) in enumerate(src):
                            nc.sync.dma_start(
                                hT_next[c][:, hh * Nh:(hh + 1) * Nh],
                                ag_q[hh][r, rb * P:(rb + 1) * P, :])
                    # col norms, quarter at a time: ssq replicated onto all
                    # 128 partitions via ones-matrix matmul, then wide
                    # max/sqrt/recip (no serial 1-partition ops)
                    for ic in range(NI):
                        sl = slice(ic * CW, (ic + 1) * CW)
                        psw = psum_p.tile([P, CW], FP, tag="ps", name="psw")
                        for c in range(6):
                            sq = sq_p.tile([P, CW], BF, tag="sq", name="sq")
                            nc.vector.tensor_tensor(
                                sq[:], hT_next[c][:, sl], hT_next[c][:, sl],
                                ALU.mult)
                            nc.tensor.matmul(psw[:], ones128[:], sq[:],
                                             start=(c == 0), stop=(c == 5))
                        nrm = rn_p.tile([P, CW], FP, tag="nrm", name="nrm")
                        nc.vector.tensor_scalar_max(nrm[:], psw[:],
                                                    EPS * EPS)
                        nc.scalar.activation(nrm[:], nrm[:], AF.Sqrt)
                        rn = rn_p.tile([P, CW], FP, tag="rn", name="rn")
                        nc.vector.reciprocal_approx_fast(rn[:], nrm[:])
                        for c in range(6):
                            nc.vector.tensor_tensor(hT_next[c][:, sl],
                                                    hT_next[c][:, sl],
                                                    rn[:], ALU.mult)
                            tmp = sq_p.tile([P, CW], F16, tag="lk", bufs=3,
                                            name="tmp")
                            nc.scalar.mul(tmp[:], hT_next[c][:, sl], LEAK)
                            nc.vector.tensor_max(hT_next[c][:, sl],
                                                 hT_next[c][:, sl], tmp[:])
                    hT = hT_next
                else:
                    # ---- l2: only drug rows N-2, N-1 ----
                    pb2 = psum_p.tile([2, DOUT], FP, tag="ps", name="pb2")
                    for jj in range(NT):
                        nc.tensor.matmul(pb2[:],
                                         invTcols[:, jj * 2:(jj + 1) * 2],
                                         a_sb[jj][:],
                                         start=(jj == 0), stop=(jj == NT - 1))
                    # full bias chunk for the two drug rows (local, no AG)
                    pbias = psum_p.tile([2, 2 * BW], FP, tag="ps",
                                        name="pbias")
                    for d in range(ND):
                        nc.tensor.matmul(
                            pbias[:], hT[d][:, N - 2:N],
                            wb_t[:, d * 2 * BW:(d + 1) * 2 * BW],
                            start=(d == 0), stop=(d == ND - 1))
                    stage2 = dram_p.tile([2, DOUT], F16, tag="stl2", bufs=1,
                                         name="stage2")
                    s2 = misc_p.tile([2, DOUT], F16, tag="s2", bufs=1,
                                     name="s2")
                    nc.vector.tensor_copy(s2[:], pb2[:])
                    nc.scalar.dma_start(stage2[:], s2[:])
                    ag2 = dram_p.tile([2, 2, DOUT], F16, tag="ag2", bufs=1,
                                      name="ag2")
                    nc.gpsimd.collective_compute(
                        "AllGather", ALU.bypass, replica_groups=groups,
                        ins=[stage2.opt()], outs=[ag2.opt()])
                    drh = misc_p.tile([2, D3], F16, tag="drh", bufs=1,
                                      name="drh")
                    nc.sync.dma_start(drh[:, 0:DOUT], ag2[0, :, :])
                    nc.sync.dma_start(drh[:, DOUT:2 * DOUT], ag2[1, :, :])
                    nc.scalar.copy(drh[:, 2 * DOUT:D3], pbias[:])
                    # normalize + leaky in fp32
                    dr = misc_p.tile([2, D3], FP, tag="dr", bufs=1, name="dr")
                    dsq = misc_p.tile([2, D3], FP, tag="dsq", bufs=1,
                                      name="dsq")
                    nc.vector.tensor_tensor(dsq[:], drh[:], drh[:], ALU.mult)
                    dssq = misc_p.tile([2, 1], FP, tag="dssq", bufs=1,
                                       name="dssq")
                    nc.vector.tensor_reduce(dssq[:], dsq[:], AxisListType.X,
                                            ALU.add)
                    dnrm = misc_p.tile([2, 1], FP, tag="dnrm", bufs=1,
                                       name="dnrm")
                    nc.scalar.activation(dnrm[:], dssq[:], AF.Sqrt)
                    nc.vector.tensor_scalar_max(dnrm[:], dnrm[:], EPS)
                    drn = misc_p.tile([2, 1], FP, tag="drn", bufs=1,
                                      name="drn")
                    nc.vector.reciprocal_approx_fast(drn[:], dnrm[:])
                    nc.vector.tensor_scalar(dr[:], drh[:], drn[:], None,
                                            ALU.mult)
                    nc.scalar.mul(dsq[:], dr[:], LEAK)
                    nc.vector.tensor_max(dr[:], dr[:], dsq[:])

            # ---- head: ypred = (a P1 P2) . (b P1)  (fp32, tiny) ----
            dT = misc_p.tile([P, ND3 * 2], FP, tag="dT", bufs=1)
            for d in range(ND3):
                pt = psum_p.tile([P, 2], FP, tag="ps", name="pth")
                nc.tensor.transpose(pt[:], dr[:, d * P:(d + 1) * P],
                                    ident32[0:2, 0:2])
                nc.vector.tensor_copy(dT[:, d * 2:(d + 1) * 2], pt[:])
            pw = psum_p.tile([P, 2], FP, tag="ps", name="pw")
            for d in range(ND3):
                nc.tensor.matmul(pw[:], p1_t[:, d * DEC:(d + 1) * DEC],
                                 dT[:, d * 2:(d + 1) * 2],
                                 start=(d == 0), stop=(d == ND3 - 1))
            w_sb = misc_p.tile([P, 2], FP, tag="w_sb", bufs=1)
            nc.vector.tensor_copy(w_sb[:], pw[:])
            ptt = psum_p.tile([P, 1], FP, tag="ps", name="ptt")
            nc.tensor.matmul(ptt[:], p2_t[:], w_sb[:, 0:1], start=True,
                             stop=True)
            t_sb = misc_p.tile([P, 1], FP, tag="t_sb", bufs=1)
            nc.vector.tensor_copy(t_sb[:], ptt[:])
            py = psum_p.tile([1, 1], FP, tag="ps", name="py")
            nc.tensor.matmul(py[:], t_sb[:], w_sb[:, 1:2], start=True,
                             stop=True)
            y_sb = misc_p.tile([1, 1], FP, tag="y_sb", bufs=1)
            nc.vector.tensor_copy(y_sb[:], py[:])
            nc.sync.dma_start(y_d.ap(), y_sb[:])

    nc.compile()
    return nc


# ---------------------------------------------------------------------------
# Host-side input prep
# ---------------------------------------------------------------------------

def make_in_maps(inputs: dict, n_cores: int):
    """Per-core input dicts. Core 2b = up path of batch b, 2b+1 = down."""
    f32 = lambda a: np.ascontiguousarray(np.asarray(a, dtype=np.float32))
    f16 = lambda a: np.ascontiguousarray(
        np.asarray(a, dtype=np.float32).astype(np.float16))

    def bake(w):
        w = np.array(w, dtype=np.float32, copy=True)
        w[-2:, :] = 1.0
        w[:, -2:] = 1.0
        return w

    maps = []
    for c in range(n_cores):
        b, down = divmod(c, 2)
        m = {
            "xT": f16(np.asarray(inputs["x"][b]).T),
            "p1": f32(inputs["parameter1"]),
            "p2": f32(inputs["parameter2"]),
        }
        for l in range(3):
            m[f"w{l}b"] = f16(inputs[f"l{l}_bias"])  # full, both cores
        if not down:
            m["adj"] = f16(inputs["adj"][b])
            m["invT"] = f16(np.asarray(inputs["up_inv_deg"][b]).T)
            for l in range(3):
                m[f"w{l}a"] = f16(bake(inputs[f"l{l}_up_adj_w"]))
                m[f"w{l}c"] = f16(inputs[f"l{l}_up_w"])
        else:
            m["adj"] = f16(np.asarray(inputs["adj"][b]).T)
            m["invT"] = f16(np.asarray(inputs["down_inv_deg"][b]).T)
            for l in range(3):
                m[f"w{l}a"] = f16(bake(inputs[f"l{l}_down_adj_w"]).T)
                m[f"w{l}c"] = f16(inputs[f"l{l}_down_w"])
        maps.append(m)
    return maps


_nc_cache = {}


def _get_program(n_cores, N):
    key = (n_cores, N)
    if key not in _nc_cache:
        _nc_cache[key] = build_program(n_cores, N)
    return _nc_cache[key]


def kernel(**inputs) -> np.ndarray:
    n_cores = 8
    nc = _get_program(n_cores, N_FULL)
    in_maps = make_in_maps(inputs, n_cores)
    res = run_bass_kernel_spmd(nc, in_maps, core_ids=list(range(n_cores)))
    out = np.zeros((B, 1), dtype=np.float32)
    for b in range(B):
        out[b, 0] = res.results[2 * b]["ypred"][0, 0]
    return out
